# revision 18
# baseline (speedup 1.0000x reference)
"""Trainium2 Bass kernel for nn_AdjacencyGenerator (gnn_message_passing).

Math note (verified against the reference to ~5e-7 rel err):
  The reference builds att = softmax(..., axis=1) over an [E, E, D] tensor and
  then contracts it with einsum('ijk,il->ikl', att, Wh).  Since the j index
  appears only in att and softmax normalizes over j, sum_j att[i,j,k] == 1
  exactly, so h_prime[i,k,l] == Wh[i,l].  Every op after that point is
  row-wise over the [E*D, D] view, and row i*D+k of that view is Wh[i,:]
  independent of k.  The whole attention tensor therefore cancels and the
  output is a per-edge scalar o[i] = f(Wh[i,:]) repeated D times.

  f is: elu -> LN(na) -> ff linear -> leaky -> LN(nf) -> wl linear -> leaky
        -> w5 linear -> +residual -> LN(fn) -> wv linear.

  Exact algebraic folds used on the host (none are approximations):
    * na_g/na_b fold into ff_w/ff_b            (LN -> Linear)
    * fn_g/fn_b fold into wv_w/wv_b            (LN -> Linear)
    * w5_b folds into the nf bias (residual) with a compensating
      correction wl_b -= wl_w @ w5_b           (kills the w5 bias matmul)
    * elu is computed as elu(x)+1 = exp(min(x,0)) + max(x,0); the +1 shift
      is constant along the normalized axis so the following LN cancels it.

  rsqrt(var+eps) is computed on the vector engine (bit-trick + 2 Newton
  steps, rel err ~5e-6) so the scalar engine only ever runs Exp — a single
  ACT table set, pre-warmed off the critical path.

Distribution: shard the E=1024 edges 128 per core across 8 NeuronCores,
data-parallel; all weights replicated.  The edge gather x[edge_index[1]] is
part of input sharding, done on the host.  All weights ship as one packed
[128, 1284] image + one [128, 384] critical image to minimize DMA issues.
"""

import numpy as np

D = 128
E = 1024
NCORES = 8
PER = E // NCORES  # 128 edges per core
EPS = 1e-5
MAGIC = 0x5F3759DF

# column offsets inside the packed weight images
# wpackA [128, 384]: W | ident | ffb(row 0)
A_W, A_ID, A_FFB = 0, 128, 256
# wpackB [128, 1284]:
B_FFWT, B_WLWT, B_W5, B_WVR, B_NFG, B_NFB, B_WLB3, B_WVB = (
    0, 128, 512, 896, 1024, 1152, 1280, 1283)
B_COLS = 1284

_CACHE = {}


class _Seq:
    """Sequential instruction emitter for one engine with semaphore tags."""

    def __init__(self, eng, sem, self_wait):
        self.eng, self.sem, self.n = eng, sem, 0
        self.self_wait = self_wait

    def emit(self, make, waits=()):
        if self.self_wait and self.n:
            self.eng.wait_ge(self.sem, self.n)
        for s, v in waits:
            self.eng.wait_ge(s, v)
        make().then_inc(self.sem, 1)
        self.n += 1
        return self.n


def _build_nc(validation=False):
    import concourse.bass as bass
    from concourse import mybir

    f32 = mybir.dt.float32
    i32 = mybir.dt.int32
    Alu = mybir.AluOpType
    Act = mybir.ActivationFunctionType

    nc = bass.Bass(detect_race_conditions=validation)
    # Same-engine back-to-back RAW needs explicit waits on real HW (the DVE
    # pipe drain alone is not sufficient — measured NaNs without them), so
    # self-waits stay on in both builds.
    validation = True

    d_xjT = nc.dram_tensor("xjt", [D, PER], f32, kind="ExternalInput")
    d_wA = nc.dram_tensor("wpacka", [128, 384], f32, kind="ExternalInput")
    d_wB = nc.dram_tensor("wpackb", [128, B_COLS], f32, kind="ExternalInput")
    d_out = nc.dram_tensor("out", [PER, D], f32, kind="ExternalOutput")

    from contextlib import ExitStack

    ctx = ExitStack()
    sb = lambda name, shape, dt=f32: ctx.enter_context(
        nc.sbuf_tensor(name, shape, dt))
    ps = lambda name, shape: ctx.enter_context(nc.psum_tensor(name, shape, f32))

    s_xjT = sb("s_xjt", [D, PER])
    s_wA = sb("s_wa", [128, 384])
    s_wB = sb("s_wb", [128, B_COLS])

    ones = sb("ones", [1, 128])
    m0 = sb("m0", [PER, D])        # min(Wh, 0)
    ex = sb("ex", [PER, D])        # exp(min(Wh, 0))
    t1 = sb("t1", [PER, D])        # elu(Wh) + 1
    t2 = sb("t2", [PER, D])        # LN1 core
    t2T = sb("t2t", [D, PER])
    lk1 = sb("lk1", [PER, D])
    t3 = sb("t3", [PER, D])        # leaky(ff out)
    u = sb("u", [PER, D])          # LN2 core
    t4a = sb("t4a", [PER, D])
    t4 = sb("t4", [PER, D])
    t4T = sb("t4t", [D, PER])
    lka = sb("lka", [128, 3, PER])
    lkb = sb("lkb", [128, 3, PER])
    y1T = sb("y1t", [128, 3, PER])
    y3 = sb("y3", [PER, D])
    y4 = sb("y4", [PER, D])        # LN3 core
    y4T = sb("y4t", [D, PER])
    o_sb = sb("o_sb", [PER, D])
    # LN statistics scratch (reused across the three LNs)
    st = sb("st", [PER, 6])
    mv = sb("mv", [PER, 2])
    xv = sb("xv", [PER, 1])        # var + eps
    nrA = sb("nra", [PER, 1])      # newton scratch / y
    nrB = sb("nrb", [PER, 1])
    nrC = sb("nrc", [PER, 1])
    magic = sb("magic", [PER, 1], i32)
    onei = sb("onei", [PER, 1], i32)
    zeroc = sb("zeroc", [PER, 1])
    scr = sb("scr", [1, 1])        # ACT warmup scratch

    p_wh = ps("p_wh", [PER, D])
    p_t2T = ps("p_t2t", [D, PER])
    p_q2 = ps("p_q2", [PER, D])
    p_t4T = ps("p_t4t", [D, PER])
    p_y1T = ps("p_y1t", [128, 3, PER])
    p_y2 = ps("p_y2", [PER, D])
    p_y4T = ps("p_y4t", [D, PER])
    p_o = ps("p_o", [PER, D])

    dsem_x = ctx.enter_context(nc.semaphore("dsem_x"))
    dsem_a = ctx.enter_context(nc.semaphore("dsem_a"))
    dsem_b = ctx.enter_context(nc.semaphore("dsem_b"))
    psem = ctx.enter_context(nc.semaphore("psem"))
    vsem = ctx.enter_context(nc.semaphore("vsem"))
    asem = ctx.enter_context(nc.semaphore("asem"))

    # ---- vector op index bookkeeping ------------------------------------
    # fixed prologue: ones, magic, onei, zeroc  (1..4)
    V_ONES = 1
    V_SETUP = 4

    def ln_indices(base):
        # bn_stats, bn_aggr, xv, shift, sub, then 2 newton iters (4 ops each),
        # then apply-ts => base+1 .. base+14
        return dict(stats=base + 1, aggr=base + 2, rstd=base + 13,
                    apply=base + 14)

    V_M0 = V_SETUP + 1            # 4
    V_T1 = V_SETUP + 2            # 5
    LN1 = ln_indices(V_T1)        # 6..21 ; apply -> t2 = 21
    V_T2 = LN1["apply"]
    V_T2T = V_T2 + 1              # copy
    V_LK1A = V_T2T + 1
    V_T3 = V_LK1A + 1
    LN2 = ln_indices(V_T3)
    V_U = LN2["apply"]
    V_T4A = V_U + 1
    V_T4 = V_T4A + 1
    V_T4T = V_T4 + 1
    # per-chunk leaky for wl output: 3 ops per chunk
    V_Y1T = [V_T4T + 3 * (c + 1) for c in range(3)]
    V_Y3 = V_Y1T[2] + 1
    LN3 = ln_indices(V_Y3)
    V_Y4 = LN3["apply"]
    V_Y4T = V_Y4 + 1
    V_OSB = V_Y4T + 1

    # ---- PE op indices ---------------------------------------------------
    P_WH = 1
    P_Q2B = 2
    P_T2T = 3
    P_Q2 = 4
    P_T4T = 5
    P_WL = [6, 7, 8]
    P_Y2 = [9, 10, 11]
    P_Y4T = 12
    P_O = 13

    A_EXW = 1   # exp table warm
    A_EX = 2

    with nc.Block() as block:

        @block.sync
        def _(sync):
            sync.dma_start(out=s_xjT[:, :], in_=d_xjT[:, :]).then_inc(dsem_x, 16)
            sync.dma_start(out=s_wA[:, :], in_=d_wA[:, :]).then_inc(dsem_a, 16)
            sync.dma_start(out=s_wB[:, :], in_=d_wB[:, :]).then_inc(dsem_b, 16)
            sync.wait_ge(vsem, V_OSB)
            sync.dma_start(out=d_out[:, :], in_=o_sb[:, :]).then_inc(dsem_x, 16)
            sync.wait_ge(dsem_x, 32)

        @block.scalar
        def _(se):
            A = _Seq(se, asem, validation)
            # pre-warm the Exp table set off the critical path
            A.emit(lambda: se.activation(out=scr[:, :], in_=ones[0:1, 0:1],
                                         func=Act.Exp),
                   waits=[(vsem, V_ONES)])
            A.emit(lambda: se.activation(out=ex[:, :], in_=m0[:, :],
                                         func=Act.Exp),
                   waits=[(vsem, V_M0)])

        @block.tensor
        def _(te):
            T = _Seq(te, psem, validation)
            # Wh = xj @ W
            T.emit(lambda: te.matmul(p_wh[:, :], s_xjT[:, :],
                                     s_wA[:, A_W:A_W + 128],
                                     start=True, stop=True),
                   waits=[(dsem_x, 16), (dsem_a, 16)])
            # ff bias first (no chain deps) so it hides under elu/LN1
            T.emit(lambda: te.matmul(p_q2[:, :], ones[:, :],
                                     s_wA[0:1, A_FFB:A_FFB + 128],
                                     start=True, stop=False,
                                     skip_group_check=True),
                   waits=[(vsem, V_ONES)])
            T.emit(lambda: te.transpose(p_t2T[:, :], t2[:, :],
                                        s_wA[:, A_ID:A_ID + 128]),
                   waits=[(vsem, V_T2)])
            T.emit(lambda: te.matmul(p_q2[:, :], t2T[:, :],
                                     s_wB[:, B_FFWT:B_FFWT + 128],
                                     start=False, stop=True,
                                     skip_group_check=True),
                   waits=[(vsem, V_T2T), (dsem_b, 16)])
            T.emit(lambda: te.transpose(p_t4T[:, :], t4[:, :],
                                        s_wA[:, A_ID:A_ID + 128]),
                   waits=[(vsem, V_T4)])
            for c in range(3):
                T.emit(lambda c=c: te.matmul(
                    p_y1T[:, c, :],
                    s_wB[:, B_WLWT + c * 128:B_WLWT + (c + 1) * 128],
                    t4T[:, :], start=True, stop=True),
                    waits=[(vsem, V_T4T)] if c == 0 else ())
            for c in range(3):
                T.emit(lambda c=c: te.matmul(
                    p_y2[:, :], y1T[:, c, :],
                    s_wB[:, B_W5 + c * 128:B_W5 + (c + 1) * 128],
                    start=(c == 0), stop=(c == 2)),
                    waits=[(vsem, V_Y1T[c])])
            T.emit(lambda: te.transpose(p_y4T[:, :], y4[:, :],
                                        s_wA[:, A_ID:A_ID + 128]),
                   waits=[(vsem, V_Y4)])
            T.emit(lambda: te.matmul(p_o[:, :], y4T[:, :],
                                     s_wB[:, B_WVR:B_WVR + 128],
                                     start=True, stop=True),
                   waits=[(vsem, V_Y4T)])
            assert T.n == P_O

        @block.vector
        def _(ve):
            V = _Seq(ve, vsem, validation)
            V.emit(lambda: ve.memset(ones[:, :], 1.0))
            V.emit(lambda: ve.memset(magic[:, :], MAGIC))
            V.emit(lambda: ve.memset(onei[:, :], 1))
            V.emit(lambda: ve.memset(zeroc[:, :], 0.0))
            assert V.n == V_SETUP

            V.emit(lambda: ve.tensor_scalar_min(out=m0[:, :], in0=p_wh[:, :],
                                                scalar1=0.0),
                   waits=[(psem, P_WH)])
            V.emit(lambda: ve.scalar_tensor_tensor(out=t1[:, :], in0=p_wh[:, :],
                                                   scalar=0.0, in1=ex[:, :],
                                                   op0=Alu.max, op1=Alu.add),
                   waits=[(asem, A_EX)])
            assert V.n == V_T1

            def ln_core(src, dst):
                """dst = (src - mean)/sqrt(var+eps), rstd via Newton rsqrt."""
                V.emit(lambda: ve.bn_stats(out=st[:, :], in_=src[:, :]))
                V.emit(lambda: ve.bn_aggr(out=mv[:, :], in_=st[:, :]))
                V.emit(lambda: ve.tensor_scalar_add(out=xv[:, :],
                                                    in0=mv[:, 1:2],
                                                    scalar1=EPS))
                V.emit(lambda: ve.tensor_tensor(
                    out=nrB.bitcast(i32)[:, :], in0=xv.bitcast(i32)[:, :],
                    in1=onei[:, :], op=Alu.arith_shift_right))
                V.emit(lambda: ve.tensor_tensor(
                    out=nrA.bitcast(i32)[:, :], in0=magic[:, :],
                    in1=nrB.bitcast(i32)[:, :], op=Alu.subtract))
                for _ in range(2):  # Newton: y = y*(1.5 - 0.5*x*y^2)
                    V.emit(lambda: ve.tensor_mul(out=nrB[:, :], in0=nrA[:, :],
                                                 in1=nrA[:, :]))
                    V.emit(lambda: ve.tensor_mul(out=nrC[:, :], in0=nrB[:, :],
                                                 in1=xv[:, :]))
                    V.emit(lambda: ve.tensor_scalar(out=nrC[:, :],
                                                    in0=nrC[:, :],
                                                    scalar1=-0.5, scalar2=1.5,
                                                    op0=Alu.mult, op1=Alu.add))
                    V.emit(lambda: ve.tensor_mul(out=nrA[:, :], in0=nrA[:, :],
                                                 in1=nrC[:, :]))
                V.emit(lambda: ve.tensor_scalar(out=dst[:, :], in0=src[:, :],
                                                scalar1=mv[:, 0:1],
                                                scalar2=nrA[:, 0:1],
                                                op0=Alu.subtract,
                                                op1=Alu.mult))

            ln_core(t1, t2)
            assert V.n == V_T2
            V.emit(lambda: ve.tensor_copy(out=t2T[:, :], in_=p_t2T[:, :]),
                   waits=[(psem, P_T2T)])
            # leaky(q2) = q2 - 0.8*min(q2, 0)
            V.emit(lambda: ve.tensor_scalar(out=lk1[:, :], in0=p_q2[:, :],
                                            scalar1=0.0, scalar2=0.8,
                                            op0=Alu.min, op1=Alu.mult),
                   waits=[(psem, P_Q2)])
            V.emit(lambda: ve.tensor_sub(out=t3[:, :], in0=p_q2[:, :],
                                         in1=lk1[:, :]))
            assert V.n == V_T3
            ln_core(t3, u)
            assert V.n == V_U
            # t4 = u * nf_g + (nf_b + w5_b)
            V.emit(lambda: ve.tensor_mul(out=t4a[:, :], in0=u[:, :],
                                         in1=s_wB[:, B_NFG:B_NFG + 128]))
            V.emit(lambda: ve.tensor_add(out=t4[:, :], in0=t4a[:, :],
                                         in1=s_wB[:, B_NFB:B_NFB + 128]))
            V.emit(lambda: ve.tensor_copy(out=t4T[:, :], in_=p_t4T[:, :]),
                   waits=[(psem, P_T4T)])
            assert V.n == V_T4T
            # per-chunk leaky+bias on the transposed wl output:
            #   y1T_c = (mm_c + b_c) - 0.8*min(mm_c + b_c, 0)
            # NB: all three chunks share one PSUM bank, so the DVE must not
            # read it while the PE is still writing it (same-bank PE-W/DVE-R
            # is fatal) — wait for the last wl matmul before the first read.
            for c in range(3):
                wlb_c = s_wB[:, B_WLB3 + c:B_WLB3 + c + 1]
                V.emit(lambda c=c, wlb_c=wlb_c: ve.tensor_scalar(
                    out=lka[:, c, :], in0=p_y1T[:, c, :],
                    scalar1=wlb_c, scalar2=zeroc[:, 0:1],
                    op0=Alu.add, op1=Alu.min),
                    waits=[(psem, P_WL[2])] if c == 0 else ())
                V.emit(lambda c=c: ve.scalar_tensor_tensor(
                    out=lkb[:, c, :], in0=lka[:, c, :], scalar=-0.8,
                    in1=p_y1T[:, c, :], op0=Alu.mult, op1=Alu.add))
                V.emit(lambda c=c, wlb_c=wlb_c: ve.tensor_scalar_add(
                    out=y1T[:, c, :], in0=lkb[:, c, :], scalar1=wlb_c))
                assert V.n == V_Y1T[c]
            V.emit(lambda: ve.tensor_add(out=y3[:, :], in0=p_y2[:, :],
                                         in1=t4[:, :]),
                   waits=[(psem, P_Y2[2])])
            assert V.n == V_Y3
            ln_core(y3, y4)
            assert V.n == V_Y4
            V.emit(lambda: ve.tensor_copy(out=y4T[:, :], in_=p_y4T[:, :]),
                   waits=[(psem, P_Y4T)])
            # out = p_o + wvb (wv bias is a scalar constant per partition)
            V.emit(lambda: ve.tensor_scalar_add(
                out=o_sb[:, :], in0=p_o[:, :],
                scalar1=s_wB[:, B_WVB:B_WVB + 1]),
                waits=[(psem, P_O)])
            assert V.n == V_OSB

    return nc, ctx


def _get_nc(validation=False):
    key = "ncv" if validation else "nc"
    if key not in _CACHE:
        _CACHE[key] = _build_nc(validation)
    return _CACHE[key][0]


def _prep_in_maps(inputs):
    """Host-side sharding + exact algebraic weight folding + packing."""
    g = lambda k: np.asarray(inputs[k], dtype=np.float64)
    x = g("x")
    ei = np.asarray(inputs["edge_index"]).astype(np.int64)
    W = g("W")
    ff_w, ff_b = g("ff_w"), g("ff_b")
    na_g, na_b = g("na_g"), g("na_b")
    nf_g, nf_b = g("nf_g"), g("nf_b")
    wl_w, wl_b = g("wl_w"), g("wl_b")
    w5_w, w5_b = g("w5_w"), g("w5_b")
    fn_g, fn_b = g("fn_g"), g("fn_b")
    wv_w, wv_b = g("wv_w"), g("wv_b")

    xj = x[ei[1]]                           # [E, D] gather on host
    ffw_eff = ff_w * na_g[None, :]          # fold LN(na) scale into ff
    ffb_eff = ff_b + ff_w @ na_b            # fold LN(na) bias into ff
    wv_eff = wv_w[0] * fn_g                 # fold LN(fn) scale into wv
    wvb_eff = wv_b[0] + wv_w[0] @ fn_b      # fold LN(fn) bias into wv
    wlb_eff = wl_b - wl_w @ w5_b            # compensate w5_b folded into t4
    nfb_eff = nf_b + w5_b                   # w5 bias rides the residual

    wA = np.zeros((128, 384), np.float64)
    wA[:, A_W:A_W + 128] = W
    wA[:, A_ID:A_ID + 128] = np.eye(128)
    wA[0, A_FFB:A_FFB + 128] = ffb_eff

    wB = np.zeros((128, B_COLS), np.float64)
    wB[:, B_FFWT:B_FFWT + 128] = ffw_eff.T
    wB[:, B_WLWT:B_WLWT + 384] = wl_w.T
    wB[:, B_W5:B_W5 + 384] = w5_w.T.reshape(3, 128, 128).transpose(
        1, 0, 2).reshape(128, 384)
    wB[:, B_WVR:B_WVR + 128] = wv_eff[:, None]
    wB[:, B_NFG:B_NFG + 128] = nf_g[None, :]
    wB[:, B_NFB:B_NFB + 128] = nfb_eff[None, :]
    wB[:, B_WLB3:B_WLB3 + 3] = wlb_eff.reshape(3, 128).T
    wB[:, B_WVB] = wvb_eff

    f32 = lambda a: np.ascontiguousarray(a, dtype=np.float32)
    shared = {"wpacka": f32(wA), "wpackb": f32(wB)}
    in_maps = []
    for c in range(NCORES):
        m = dict(shared)
        m["xjt"] = f32(xj[c * PER:(c + 1) * PER].T)
        in_maps.append(m)
    return in_maps


def kernel(**inputs) -> np.ndarray:
    from concourse.bass_utils import run_bass_kernel_spmd

    nc = _get_nc()
    in_maps = _prep_in_maps(inputs)
    res = run_bass_kernel_spmd(nc, in_maps, core_ids=list(range(NCORES)))
    return np.concatenate(
        [np.asarray(res.results[c]["out"]).reshape(-1) for c in range(NCORES)]
    )


# revision 26
# speedup vs baseline: 1.1625x; 1.1625x over previous
"""Trainium2 Bass kernel for nn_AdjacencyGenerator (gnn_message_passing).

Math note (verified against the reference to ~5e-7 rel err):
  The reference builds att = softmax(..., axis=1) over an [E, E, D] tensor and
  then contracts it with einsum('ijk,il->ikl', att, Wh).  Since the j index
  appears only in att and softmax normalizes over j, sum_j att[i,j,k] == 1
  exactly, so h_prime[i,k,l] == Wh[i,l].  Every op after that point is
  row-wise over the [E*D, D] view, and row i*D+k of that view is Wh[i,:]
  independent of k.  The whole attention tensor therefore cancels and the
  output is a per-edge scalar o[i] = f(Wh[i,:]) repeated D times.

  f is: elu -> LN(na) -> ff linear -> leaky -> LN(nf) -> wl linear -> leaky
        -> w5 linear -> +residual -> LN(fn) -> wv linear.

  Exact algebraic folds used on the host (none are approximations):
    * na_g/na_b fold into ff_w/ff_b            (LN -> Linear)
    * fn_g/fn_b fold into wv_w/wv_b            (LN -> Linear)
    * w5_b folds into the nf bias (residual) with a compensating
      correction wl_b -= wl_w @ w5_b           (kills the w5 bias matmul)
    * elu is computed as elu(x)+1 = exp(min(x,0)) + max(x,0); the +1 shift
      is constant along the normalized axis so the following LN cancels it.

  rsqrt(var+eps) is computed on the vector engine (bit-trick + 2 Newton
  steps, rel err ~5e-6) so the scalar engine only ever runs Exp — a single
  ACT table set, pre-warmed off the critical path.

Distribution: shard the E=1024 edges 128 per core across 8 NeuronCores,
data-parallel; all weights replicated.  The edge gather x[edge_index[1]] is
part of input sharding, done on the host.  All weights ship as one packed
[128, 1284] image + one [128, 384] critical image to minimize DMA issues.
"""

import numpy as np

D = 128
E = 1024
NCORES = 8
PER = E // NCORES  # 128 edges per core
EPS = 1e-5
MAGIC = 0x5F3759DF

# column offsets inside the packed weight images
# wpackA [128, 384]: W | ident | ffb(row 0)
A_W, A_ID, A_FFB = 0, 128, 256
# wpackB [128, 1284]:
B_FFWT, B_WLWT, B_W5, B_WVR, B_NFG, B_NFB, B_WLB3, B_WVB = (
    0, 128, 512, 896, 1024, 1152, 1280, 1283)
B_COLS = 1284

_CACHE = {}


class _Seq:
    """Sequential instruction emitter for one engine with semaphore tags.

    attach=True (single-instruction ops, DVE/ACT): waits ride on the
    instruction's own sync_info — no separate wait instruction.
    attach=False (multi-instruction ops like matmul, and DMA): standalone
    wait instructions are emitted first so they gate the whole group.
    """

    def __init__(self, eng, sem, all_self_waits, attach=False):
        self.eng, self.sem, self.n = eng, sem, 0
        self.all_self_waits = all_self_waits
        self.attach = attach

    def emit(self, make, waits=(), self_wait=False):
        allw = list(waits)
        if (self_wait or self.all_self_waits) and self.n:
            allw.append((self.sem, self.n))
        if self.attach and allw:
            # at most one wait fits on an instruction's sync_info;
            # spill the rest as standalone waits
            for s, v in allw[:-1]:
                self.eng.wait_ge(s, v)
            inst = make()
            inst._wait_ge(*allw[-1])
        else:
            for s, v in allw:
                self.eng.wait_ge(s, v)
            inst = make()
        inst.then_inc(self.sem, 1)
        self.n += 1
        return self.n


def _build_nc(validation=False):
    import concourse.bass as bass
    from concourse import bacc, mybir

    f32 = mybir.dt.float32
    i32 = mybir.dt.int32
    Alu = mybir.AluOpType
    Act = mybir.ActivationFunctionType

    nc = bass.Bass(detect_race_conditions=validation)
    # Same-engine back-to-back RAW needs explicit waits on real HW (the DVE
    # pipe drain alone is not sufficient — measured NaNs without them), so
    # self-waits stay on in both builds.
    validation = True

    d_xjT = nc.dram_tensor("xjt", [D, PER], f32, kind="ExternalInput")
    d_wA = nc.dram_tensor("wpacka", [128, 384], f32, kind="ExternalInput")
    d_wB = nc.dram_tensor("wpackb", [128, B_COLS], f32, kind="ExternalInput")
    d_out = nc.dram_tensor("out", [PER, D], f32, kind="ExternalOutput")

    from contextlib import ExitStack

    ctx = ExitStack()
    sb = lambda name, shape, dt=f32: ctx.enter_context(
        nc.sbuf_tensor(name, shape, dt))
    ps = lambda name, shape: ctx.enter_context(nc.psum_tensor(name, shape, f32))

    s_xjT = sb("s_xjt", [D, PER])
    s_wA = sb("s_wa", [128, 384])
    s_wB = sb("s_wb", [128, B_COLS])

    ones = sb("ones", [1, 128])
    m0 = sb("m0", [PER, D])        # min(Wh, 0)
    ex = sb("ex", [PER, D])        # exp(min(Wh, 0))
    t1 = sb("t1", [PER, D])        # elu(Wh) + 1
    t2 = sb("t2", [PER, D])        # LN1 core
    t2T = sb("t2t", [D, PER])
    lk1 = sb("lk1", [PER, D])
    t3 = sb("t3", [PER, D])        # leaky(ff out)
    u = sb("u", [PER, D])          # LN2 core
    t4a = sb("t4a", [PER, D])
    t4 = sb("t4", [PER, D])
    t4T = sb("t4t", [D, PER])
    lka = sb("lka", [128, 3, PER])
    lkb = sb("lkb", [128, 3, PER])
    y1T = sb("y1t", [128, 3, PER])
    y3 = sb("y3", [PER, D])
    y4 = sb("y4", [PER, D])        # LN3 core
    y4T = sb("y4t", [D, PER])
    o_sb = sb("o_sb", [PER, D])
    # LN statistics scratch (reused across the three LNs)
    st = sb("st", [PER, 6])
    mv = sb("mv", [PER, 2])
    xv = sb("xv", [PER, 1])        # var + eps
    nrA = sb("nra", [PER, 1])      # newton scratch / y
    nrB = sb("nrb", [PER, 1])
    nrC = sb("nrc", [PER, 1])
    magic = sb("magic", [PER, 1], i32)
    onei = sb("onei", [PER, 1], i32)
    zeroc = sb("zeroc", [PER, 1])
    scr = sb("scr", [1, 1])        # ACT warmup scratch

    p_wh = ps("p_wh", [PER, D])
    p_t2T = ps("p_t2t", [D, PER])
    p_q2 = ps("p_q2", [PER, D])
    p_t4T = ps("p_t4t", [D, PER])
    p_y1T = ps("p_y1t", [128, 3, PER])
    p_y2 = ps("p_y2", [PER, D])
    p_y4T = ps("p_y4t", [D, PER])
    p_o = ps("p_o", [PER, D])

    dsem_x = ctx.enter_context(nc.semaphore("dsem_x"))
    dsem_a = ctx.enter_context(nc.semaphore("dsem_a"))
    dsem_b = ctx.enter_context(nc.semaphore("dsem_b"))
    psem = ctx.enter_context(nc.semaphore("psem"))
    vsem = ctx.enter_context(nc.semaphore("vsem"))
    asem = ctx.enter_context(nc.semaphore("asem"))

    # ---- vector op index bookkeeping ------------------------------------
    # fixed prologue: ones, magic, onei, zeroc  (1..4)
    V_ONES = 1
    V_SETUP = 4

    def ln_indices(base):
        # bn_stats, bn_aggr, xv, shift, sub, then 2 newton iters (4 ops each),
        # then apply-ts => base+1 .. base+14
        return dict(stats=base + 1, aggr=base + 2, rstd=base + 13,
                    apply=base + 14)

    V_M0 = V_SETUP + 1            # 4
    V_T1 = V_SETUP + 2            # 5
    LN1 = ln_indices(V_T1)        # 6..21 ; apply -> t2 = 21
    V_T2 = LN1["apply"]
    V_T2T = V_T2 + 1              # copy
    V_LK1A = V_T2T + 1
    V_T3 = V_LK1A + 1
    LN2 = ln_indices(V_T3)
    V_U = LN2["apply"]
    V_T4A = V_U + 1
    V_T4 = V_T4A + 1
    V_T4T = V_T4 + 1
    # per-chunk leaky for wl output: 3 ops per chunk
    V_Y1T = [V_T4T + 3 * (c + 1) for c in range(3)]
    V_Y3 = V_Y1T[2] + 1
    LN3 = ln_indices(V_Y3)
    V_Y4 = LN3["apply"]
    V_Y4T = V_Y4 + 1
    V_OSB = V_Y4T + 1

    # ---- PE op indices ---------------------------------------------------
    P_WH = 1
    P_Q2B = 2
    P_T2T = 3
    P_Q2 = 4
    P_T4T = 5
    P_WL = [6, 7, 8]
    P_Y2 = [9, 10, 11]
    P_Y4T = 12
    P_O = 13

    A_EXW = 1   # exp table warm
    A_EX = 2

    with nc.Block() as block:

        @block.sync
        def _(sync):
            sync.dma_start(out=s_xjT[:, :], in_=d_xjT[:, :]).then_inc(dsem_x, 16)
            sync.dma_start(out=s_wA[:, :], in_=d_wA[:, :]).then_inc(dsem_a, 16)
            sync.dma_start(out=s_wB[:, :], in_=d_wB[:, :]).then_inc(dsem_b, 16)
            sync.wait_ge(vsem, V_OSB)
            sync.dma_start(out=d_out[:, :], in_=o_sb[:, :]).then_inc(dsem_x, 16)
            sync.wait_ge(dsem_x, 32)

        @block.scalar
        def _(se):
            A = _Seq(se, asem, validation, attach=True)
            # pre-warm the Exp table set off the critical path
            A.emit(lambda: se.activation(out=scr[:, :], in_=ones[0:1, 0:1],
                                         func=Act.Exp),
                   waits=[(vsem, V_ONES)])
            A.emit(lambda: se.activation(out=ex[:, :], in_=m0[:, :],
                                         func=Act.Exp),
                   waits=[(vsem, V_M0)])

        @block.tensor
        def _(te):
            T = _Seq(te, psem, validation)
            # Wh = xj @ W
            T.emit(lambda: te.matmul(p_wh[:, :], s_xjT[:, :],
                                     s_wA[:, A_W:A_W + 128],
                                     start=True, stop=True),
                   waits=[(dsem_x, 16), (dsem_a, 16)])
            # ff bias first (no chain deps) so it hides under elu/LN1
            T.emit(lambda: te.matmul(p_q2[:, :], ones[:, :],
                                     s_wA[0:1, A_FFB:A_FFB + 128],
                                     start=True, stop=False,
                                     skip_group_check=True),
                   waits=[(vsem, V_ONES)])
            T.emit(lambda: te.transpose(p_t2T[:, :], t2[:, :],
                                        s_wA[:, A_ID:A_ID + 128]),
                   waits=[(vsem, V_T2)])
            T.emit(lambda: te.matmul(p_q2[:, :], t2T[:, :],
                                     s_wB[:, B_FFWT:B_FFWT + 128],
                                     start=False, stop=True,
                                     skip_group_check=True),
                   waits=[(vsem, V_T2T), (dsem_b, 16)])
            T.emit(lambda: te.transpose(p_t4T[:, :], t4[:, :],
                                        s_wA[:, A_ID:A_ID + 128]),
                   waits=[(vsem, V_T4)])
            for c in range(3):
                T.emit(lambda c=c: te.matmul(
                    p_y1T[:, c, :],
                    s_wB[:, B_WLWT + c * 128:B_WLWT + (c + 1) * 128],
                    t4T[:, :], start=True, stop=True),
                    waits=[(vsem, V_T4T)] if c == 0 else ())
            for c in range(3):
                T.emit(lambda c=c: te.matmul(
                    p_y2[:, :], y1T[:, c, :],
                    s_wB[:, B_W5 + c * 128:B_W5 + (c + 1) * 128],
                    start=(c == 0), stop=(c == 2)),
                    waits=[(vsem, V_Y1T[c])])
            T.emit(lambda: te.transpose(p_y4T[:, :], y4[:, :],
                                        s_wA[:, A_ID:A_ID + 128]),
                   waits=[(vsem, V_Y4)])
            T.emit(lambda: te.matmul(p_o[:, :], y4T[:, :],
                                     s_wB[:, B_WVR:B_WVR + 128],
                                     start=True, stop=True),
                   waits=[(vsem, V_Y4T)])
            assert T.n == P_O

        @block.vector
        def _(ve):
            V = _Seq(ve, vsem, validation, attach=True)
            V.emit(lambda: ve.memset(ones[:, :], 1.0))
            V.emit(lambda: ve.memset(magic[:, :], MAGIC))
            V.emit(lambda: ve.memset(onei[:, :], 1))
            V.emit(lambda: ve.memset(zeroc[:, :], 0.0))
            assert V.n == V_SETUP

            V.emit(lambda: ve.tensor_scalar_min(out=m0[:, :], in0=p_wh[:, :],
                                                scalar1=0.0),
                   waits=[(psem, P_WH)])
            V.emit(lambda: ve.scalar_tensor_tensor(out=t1[:, :], in0=p_wh[:, :],
                                                   scalar=0.0, in1=ex[:, :],
                                                   op0=Alu.max, op1=Alu.add),
                   waits=[(asem, A_EX)])
            assert V.n == V_T1

            def ln_core(src, dst):
                """dst = (src - mean)/sqrt(var+eps), rstd via Newton rsqrt."""
                V.emit(lambda: ve.bn_stats(out=st[:, :], in_=src[:, :]))
                V.emit(lambda: ve.bn_aggr(out=mv[:, :], in_=st[:, :]))
                V.emit(lambda: ve.tensor_scalar_add(out=xv[:, :],
                                                    in0=mv[:, 1:2],
                                                    scalar1=EPS))
                V.emit(lambda: ve.tensor_tensor(
                    out=nrB.bitcast(i32)[:, :], in0=xv.bitcast(i32)[:, :],
                    in1=onei[:, :], op=Alu.arith_shift_right))
                V.emit(lambda: ve.tensor_tensor(
                    out=nrA.bitcast(i32)[:, :], in0=magic[:, :],
                    in1=nrB.bitcast(i32)[:, :], op=Alu.subtract))
                for _ in range(2):  # Newton: y = y*(1.5 - 0.5*x*y^2)
                    V.emit(lambda: ve.tensor_mul(out=nrB[:, :], in0=nrA[:, :],
                                                 in1=nrA[:, :]))
                    V.emit(lambda: ve.tensor_mul(out=nrC[:, :], in0=nrB[:, :],
                                                 in1=xv[:, :]))
                    V.emit(lambda: ve.tensor_scalar(out=nrC[:, :],
                                                    in0=nrC[:, :],
                                                    scalar1=-0.5, scalar2=1.5,
                                                    op0=Alu.mult, op1=Alu.add))
                    V.emit(lambda: ve.tensor_mul(out=nrA[:, :], in0=nrA[:, :],
                                                 in1=nrC[:, :]))
                # scalar operands are latched at dispatch, before the prior
                # op's pipe drain — nrA was written one op ago, so this one
                # genuinely needs the same-engine wait
                V.emit(lambda: ve.tensor_scalar(out=dst[:, :], in0=src[:, :],
                                                scalar1=mv[:, 0:1],
                                                scalar2=nrA[:, 0:1],
                                                op0=Alu.subtract,
                                                op1=Alu.mult),
                       self_wait=True)

            ln_core(t1, t2)
            assert V.n == V_T2
            V.emit(lambda: ve.tensor_copy(out=t2T[:, :], in_=p_t2T[:, :]),
                   waits=[(psem, P_T2T)])
            # leaky(q2) = q2 - 0.8*min(q2, 0)
            V.emit(lambda: ve.tensor_scalar(out=lk1[:, :], in0=p_q2[:, :],
                                            scalar1=0.0, scalar2=0.8,
                                            op0=Alu.min, op1=Alu.mult),
                   waits=[(psem, P_Q2)])
            V.emit(lambda: ve.tensor_sub(out=t3[:, :], in0=p_q2[:, :],
                                         in1=lk1[:, :]))
            assert V.n == V_T3
            ln_core(t3, u)
            assert V.n == V_U
            # t4 = u * nf_g + (nf_b + w5_b)
            V.emit(lambda: ve.tensor_mul(out=t4a[:, :], in0=u[:, :],
                                         in1=s_wB[:, B_NFG:B_NFG + 128]))
            V.emit(lambda: ve.tensor_add(out=t4[:, :], in0=t4a[:, :],
                                         in1=s_wB[:, B_NFB:B_NFB + 128]))
            V.emit(lambda: ve.tensor_copy(out=t4T[:, :], in_=p_t4T[:, :]),
                   waits=[(psem, P_T4T)])
            assert V.n == V_T4T
            # per-chunk leaky+bias on the transposed wl output:
            #   y1T_c = (mm_c + b_c) - 0.8*min(mm_c + b_c, 0)
            # NB: all three chunks share one PSUM bank, so the DVE must not
            # read it while the PE is still writing it (same-bank PE-W/DVE-R
            # is fatal) — wait for the last wl matmul before the first read.
            for c in range(3):
                wlb_c = s_wB[:, B_WLB3 + c:B_WLB3 + c + 1]
                V.emit(lambda c=c, wlb_c=wlb_c: ve.tensor_scalar(
                    out=lka[:, c, :], in0=p_y1T[:, c, :],
                    scalar1=wlb_c, scalar2=zeroc[:, 0:1],
                    op0=Alu.add, op1=Alu.min),
                    waits=[(psem, P_WL[2])] if c == 0 else ())
                V.emit(lambda c=c: ve.scalar_tensor_tensor(
                    out=lkb[:, c, :], in0=lka[:, c, :], scalar=-0.8,
                    in1=p_y1T[:, c, :], op0=Alu.mult, op1=Alu.add))
                V.emit(lambda c=c, wlb_c=wlb_c: ve.tensor_scalar_add(
                    out=y1T[:, c, :], in0=lkb[:, c, :], scalar1=wlb_c))
                assert V.n == V_Y1T[c]
            V.emit(lambda: ve.tensor_add(out=y3[:, :], in0=p_y2[:, :],
                                         in1=t4[:, :]),
                   waits=[(psem, P_Y2[2])])
            assert V.n == V_Y3
            ln_core(y3, y4)
            assert V.n == V_Y4
            V.emit(lambda: ve.tensor_copy(out=y4T[:, :], in_=p_y4T[:, :]),
                   waits=[(psem, P_Y4T)])
            # out = p_o + wvb (wv bias is a scalar constant per partition)
            V.emit(lambda: ve.tensor_scalar_add(
                out=o_sb[:, :], in0=p_o[:, :],
                scalar1=s_wB[:, B_WVB:B_WVB + 1]),
                waits=[(psem, P_O)])
            assert V.n == V_OSB

    return nc, ctx


def _get_nc(validation=False):
    key = "ncv" if validation else "nc"
    if key not in _CACHE:
        _CACHE[key] = _build_nc(validation)
    return _CACHE[key][0]


def _prep_in_maps(inputs):
    """Host-side sharding + exact algebraic weight folding + packing."""
    g = lambda k: np.asarray(inputs[k], dtype=np.float64)
    x = g("x")
    ei = np.asarray(inputs["edge_index"]).astype(np.int64)
    W = g("W")
    ff_w, ff_b = g("ff_w"), g("ff_b")
    na_g, na_b = g("na_g"), g("na_b")
    nf_g, nf_b = g("nf_g"), g("nf_b")
    wl_w, wl_b = g("wl_w"), g("wl_b")
    w5_w, w5_b = g("w5_w"), g("w5_b")
    fn_g, fn_b = g("fn_g"), g("fn_b")
    wv_w, wv_b = g("wv_w"), g("wv_b")

    xj = x[ei[1]]                           # [E, D] gather on host
    ffw_eff = ff_w * na_g[None, :]          # fold LN(na) scale into ff
    ffb_eff = ff_b + ff_w @ na_b            # fold LN(na) bias into ff
    wv_eff = wv_w[0] * fn_g                 # fold LN(fn) scale into wv
    wvb_eff = wv_b[0] + wv_w[0] @ fn_b      # fold LN(fn) bias into wv
    wlb_eff = wl_b - wl_w @ w5_b            # compensate w5_b folded into t4
    nfb_eff = nf_b + w5_b                   # w5 bias rides the residual

    wA = np.zeros((128, 384), np.float64)
    wA[:, A_W:A_W + 128] = W
    wA[:, A_ID:A_ID + 128] = np.eye(128)
    wA[0, A_FFB:A_FFB + 128] = ffb_eff

    wB = np.zeros((128, B_COLS), np.float64)
    wB[:, B_FFWT:B_FFWT + 128] = ffw_eff.T
    wB[:, B_WLWT:B_WLWT + 384] = wl_w.T
    wB[:, B_W5:B_W5 + 384] = w5_w.T.reshape(3, 128, 128).transpose(
        1, 0, 2).reshape(128, 384)
    wB[:, B_WVR:B_WVR + 128] = wv_eff[:, None]
    wB[:, B_NFG:B_NFG + 128] = nf_g[None, :]
    wB[:, B_NFB:B_NFB + 128] = nfb_eff[None, :]
    wB[:, B_WLB3:B_WLB3 + 3] = wlb_eff.reshape(3, 128).T
    wB[:, B_WVB] = wvb_eff

    f32 = lambda a: np.ascontiguousarray(a, dtype=np.float32)
    shared = {"wpacka": f32(wA), "wpackb": f32(wB)}
    in_maps = []
    for c in range(NCORES):
        m = dict(shared)
        m["xjt"] = f32(xj[c * PER:(c + 1) * PER].T)
        in_maps.append(m)
    return in_maps


def kernel(**inputs) -> np.ndarray:
    from concourse.bass_utils import run_bass_kernel_spmd

    nc = _get_nc()
    in_maps = _prep_in_maps(inputs)
    res = run_bass_kernel_spmd(nc, in_maps, core_ids=list(range(NCORES)))
    return np.concatenate(
        [np.asarray(res.results[c]["out"]).reshape(-1) for c in range(NCORES)]
    )


# revision 27
# speedup vs baseline: 1.5439x; 1.3281x over previous
"""Trainium2 Bass kernel for nn_AdjacencyGenerator (gnn_message_passing).

Math note (verified against the reference to ~5e-7 rel err):
  The reference builds att = softmax(..., axis=1) over an [E, E, D] tensor and
  then contracts it with einsum('ijk,il->ikl', att, Wh).  Since the j index
  appears only in att and softmax normalizes over j, sum_j att[i,j,k] == 1
  exactly, so h_prime[i,k,l] == Wh[i,l].  Every op after that point is
  row-wise over the [E*D, D] view, and row i*D+k of that view is Wh[i,:]
  independent of k.  The whole attention tensor therefore cancels and the
  output is a per-edge scalar o[i] = f(Wh[i,:]) repeated D times.

  f is: elu -> LN(na) -> ff linear -> leaky -> LN(nf) -> wl linear -> leaky
        -> w5 linear -> +residual -> LN(fn) -> wv linear.

  Exact algebraic folds used on the host (none are approximations):
    * na_g/na_b fold into ff_w/ff_b            (LN -> Linear)
    * fn_g/fn_b fold into wv_w/wv_b            (LN -> Linear)
    * wl_b and w5_b fold jointly into the leaky shift bb and the t4 bias B,
      solving (I + wl_w @ w5_w) bb = wl_b - wl_w @ w5_b on the host — this
      removes all wl/w5 bias matmuls exactly.
    * elu is computed as elu(x)+1 = exp(min(x,0)) + max(x,0); the +1 shift
      is constant along the normalized axis so the following LN cancels it.

  rstd(var) = exp(-0.5*ln(var+eps)) on the scalar engine: ln and exp live in
  the same ACT table set, so the whole kernel uses exactly one table load,
  pre-warmed off the critical path.

Distribution: shard the E=1024 edges 128 per core across 8 NeuronCores,
data-parallel; all weights replicated.  The edge gather x[edge_index[1]] is
part of input sharding, done on the host.  Inputs ship as three packed
images: [xjT|W] (per-core), [ident|ffb], and one [128, 1284] weight image.
"""

import numpy as np

D = 128
E = 1024
NCORES = 8
PER = E // NCORES  # 128 edges per core
EPS = 1e-5

# column offsets inside the packed images
XW_XJT, XW_W = 0, 128                      # d_xw [128, 256] (per-core)
A_ID, A_FFB = 0, 128                       # d_wA [128, 256]
B_FFWT, B_WLWT, B_W5, B_WVR, B_NFG, B_NFB, B_BB3, B_WVB = (
    0, 128, 512, 896, 1024, 1152, 1280, 1283)
B_COLS = 1284

_CACHE = {}


class _Seq:
    """Sequential instruction emitter for one engine with semaphore tags.

    attach=True (single-instruction ops, DVE/ACT): one wait rides on the
    instruction's own sync_info (HW allows a single attached wait); any
    extra waits are emitted standalone.  attach=False (multi-instruction
    groups like matmul, and DMA): all waits are standalone so they gate the
    whole group.
    """

    def __init__(self, eng, sem, all_self_waits, attach=False):
        self.eng, self.sem, self.n = eng, sem, 0
        self.all_self_waits = all_self_waits
        self.attach = attach

    def emit(self, make, waits=(), self_wait=False):
        allw = list(waits)
        if (self_wait or self.all_self_waits) and self.n:
            allw.append((self.sem, self.n))
        if self.attach and allw:
            for s, v in allw[:-1]:
                self.eng.wait_ge(s, v)
            inst = make()
            inst._wait_ge(*allw[-1])
        else:
            for s, v in allw:
                self.eng.wait_ge(s, v)
            inst = make()
        inst.then_inc(self.sem, 1)
        self.n += 1
        return self.n


def _build_nc(validation=False):
    import concourse.bass as bass
    from concourse import mybir

    f32 = mybir.dt.float32
    Alu = mybir.AluOpType
    Act = mybir.ActivationFunctionType

    nc = bass.Bass(detect_race_conditions=validation)

    d_xw = nc.dram_tensor("xw", [128, 256], f32, kind="ExternalInput")
    d_wA = nc.dram_tensor("wpacka", [128, 256], f32, kind="ExternalInput")
    d_wB = nc.dram_tensor("wpackb", [128, B_COLS], f32, kind="ExternalInput")
    d_out = nc.dram_tensor("out", [PER, D], f32, kind="ExternalOutput")

    from contextlib import ExitStack

    ctx = ExitStack()
    sb = lambda name, shape, dt=f32: ctx.enter_context(
        nc.sbuf_tensor(name, shape, dt))
    ps = lambda name, shape: ctx.enter_context(nc.psum_tensor(name, shape, f32))

    s_xw = sb("s_xw", [128, 256])
    s_wA = sb("s_wa", [128, 256])
    s_wB = sb("s_wb", [128, B_COLS])

    ones = sb("ones", [1, 128])
    epsc = sb("epsc", [PER, 1])
    zeroc = sb("zeroc", [PER, 1])
    m0 = sb("m0", [PER, D])        # min(Wh, 0)
    ex = sb("ex", [PER, D])        # exp(min(Wh, 0))
    t1 = sb("t1", [PER, D])        # elu(Wh) + 1
    t2 = sb("t2", [PER, D])        # LN1 core
    t2T = sb("t2t", [D, PER])
    lk1 = sb("lk1", [PER, D])
    t3 = sb("t3", [PER, D])        # leaky(ff out)
    u = sb("u", [PER, D])          # LN2 core
    t4a = sb("t4a", [PER, D])
    t4 = sb("t4", [PER, D])
    t4T = sb("t4t", [D, PER])
    lka = sb("lka", [128, 3, PER])
    y1T = sb("y1t", [128, 3, PER])
    y3 = sb("y3", [PER, D])
    y4 = sb("y4", [PER, D])        # LN3 core
    y4T = sb("y4t", [D, PER])
    o_sb = sb("o_sb", [PER, D])
    st = sb("st", [PER, 6])        # LN scratch (reused by all three LNs)
    mv = sb("mv", [PER, 2])
    lnv = sb("lnv", [PER, 1])
    rstd = sb("rstd", [PER, 1])
    scr = sb("scr", [1, 1])        # ACT warmup scratch

    p_wh = ps("p_wh", [PER, D])
    p_t2T = ps("p_t2t", [D, PER])
    p_q2 = ps("p_q2", [PER, D])
    p_t4T = ps("p_t4t", [D, PER])
    p_y1T = ps("p_y1t", [128, 3, PER])
    p_y2 = ps("p_y2", [PER, D])
    p_y4T = ps("p_y4t", [D, PER])
    p_o = ps("p_o", [PER, D])

    dsem_x = ctx.enter_context(nc.semaphore("dsem_x"))
    dsem_a = ctx.enter_context(nc.semaphore("dsem_a"))
    dsem_b = ctx.enter_context(nc.semaphore("dsem_b"))
    psem = ctx.enter_context(nc.semaphore("psem"))
    vsem = ctx.enter_context(nc.semaphore("vsem"))
    asem = ctx.enter_context(nc.semaphore("asem"))
    gsem = ctx.enter_context(nc.semaphore("gsem"))

    # ---- vector op indices ----------------------------------------------
    V_M0, V_T1 = 1, 2
    V_ST1, V_MV1, V_T2 = 3, 4, 5
    V_T2T, V_LK1, V_T3 = 6, 7, 8
    V_ST2, V_MV2, V_U = 9, 10, 11
    V_T4A, V_T4, V_T4T = 12, 13, 14
    V_Y1T = [16, 18, 20]
    V_Y3 = 21
    V_ST3, V_MV3, V_Y4 = 22, 23, 24
    V_Y4T, V_OSB = 25, 26
    # ---- PE op indices ---------------------------------------------------
    P_WH, P_Q2B, P_T2T, P_Q2, P_T4T = 1, 2, 3, 4, 5
    P_WL = [6, 7, 8]
    P_Y2 = [9, 10, 11]
    P_Y4T, P_O = 12, 13
    # ---- ACT op indices --------------------------------------------------
    A_WARM, A_EX = 1, 2
    A_R1, A_R2, A_R3 = 4, 6, 8
    # ---- gpsimd ----------------------------------------------------------
    G_ONES, G_SETUP = 1, 3

    with nc.Block() as block:

        @block.sync
        def _(sync):
            sync.dma_start(out=s_xw[:, :], in_=d_xw[:, :]).then_inc(dsem_x, 16)
            sync.dma_start(out=s_wB[:, :], in_=d_wB[:, :]).then_inc(dsem_b, 16)
            sync.dma_start(out=s_wA[:, :], in_=d_wA[:, :]).then_inc(dsem_a, 16)
            sync.wait_ge(vsem, V_OSB)
            sync.dma_start(out=d_out[:, :], in_=o_sb[:, :]).then_inc(dsem_x, 16)
            sync.wait_ge(dsem_x, 32)

        @block.gpsimd
        def _(ge):
            ge.memset(ones[:, :], 1.0).then_inc(gsem, 1)
            ge.memset(epsc[:, :], EPS).then_inc(gsem, 1)
            ge.memset(zeroc[:, :], 0.0).then_inc(gsem, 1)

        @block.scalar
        def _(se):
            A = _Seq(se, asem, validation, attach=True)
            # pre-warm the ln/exp table set off the critical path
            A.emit(lambda: se.activation(out=scr[:, :], in_=ones[0:1, 0:1],
                                         func=Act.Ln),
                   waits=[(gsem, G_ONES)])
            A.emit(lambda: se.activation(out=ex[:, :], in_=m0[:, :],
                                         func=Act.Exp),
                   waits=[(vsem, V_M0)])
            assert A.n == A_EX
            for a_idx, v_mv in ((A_R1, V_MV1), (A_R2, V_MV2), (A_R3, V_MV3)):
                # rstd = exp(-0.5 * ln(var + eps))
                A.emit(lambda v_mv=v_mv: se.activation(
                    out=lnv[:, :], in_=mv[:, 1:2], func=Act.Ln,
                    bias=epsc[:, 0:1]),
                    waits=[(vsem, v_mv)])
                A.emit(lambda: se.activation(out=rstd[:, :], in_=lnv[:, :],
                                             func=Act.Exp, scale=-0.5),
                       self_wait=True)
                assert A.n == a_idx

        @block.tensor
        def _(te):
            T = _Seq(te, psem, validation)
            # Wh = xj @ W
            T.emit(lambda: te.matmul(p_wh[:, :], s_xw[:, XW_XJT:XW_XJT + 128],
                                     s_xw[:, XW_W:XW_W + 128],
                                     start=True, stop=True),
                   waits=[(dsem_x, 16)])
            # ff bias early (its only deps are DMA + ones memset)
            T.emit(lambda: te.matmul(p_q2[:, :], ones[:, :],
                                     s_wA[0:1, A_FFB:A_FFB + 128],
                                     start=True, stop=False,
                                     skip_group_check=True),
                   waits=[(dsem_a, 16), (dsem_b, 16), (gsem, G_ONES)])
            T.emit(lambda: te.transpose(p_t2T[:, :], t2[:, :],
                                        s_wA[:, A_ID:A_ID + 128]),
                   waits=[(vsem, V_T2)])
            T.emit(lambda: te.matmul(p_q2[:, :], t2T[:, :],
                                     s_wB[:, B_FFWT:B_FFWT + 128],
                                     start=False, stop=True,
                                     skip_group_check=True),
                   waits=[(vsem, V_T2T)])
            T.emit(lambda: te.transpose(p_t4T[:, :], t4[:, :],
                                        s_wA[:, A_ID:A_ID + 128]),
                   waits=[(vsem, V_T4)])
            for c in range(3):
                T.emit(lambda c=c: te.matmul(
                    p_y1T[:, c, :],
                    s_wB[:, B_WLWT + c * 128:B_WLWT + (c + 1) * 128],
                    t4T[:, :], start=True, stop=True),
                    waits=[(vsem, V_T4T)] if c == 0 else ())
            for c in range(3):
                T.emit(lambda c=c: te.matmul(
                    p_y2[:, :], y1T[:, c, :],
                    s_wB[:, B_W5 + c * 128:B_W5 + (c + 1) * 128],
                    start=(c == 0), stop=(c == 2)),
                    waits=[(vsem, V_Y1T[c])])
            T.emit(lambda: te.transpose(p_y4T[:, :], y4[:, :],
                                        s_wA[:, A_ID:A_ID + 128]),
                   waits=[(vsem, V_Y4)])
            T.emit(lambda: te.matmul(p_o[:, :], y4T[:, :],
                                     s_wB[:, B_WVR:B_WVR + 128],
                                     start=True, stop=True),
                   waits=[(vsem, V_Y4T)])
            assert T.n == P_O

        @block.vector
        def _(ve):
            V = _Seq(ve, vsem, validation, attach=True)
            V.emit(lambda: ve.tensor_scalar_min(out=m0[:, :], in0=p_wh[:, :],
                                                scalar1=0.0),
                   waits=[(psem, P_WH), (gsem, G_SETUP)])
            V.emit(lambda: ve.scalar_tensor_tensor(out=t1[:, :], in0=p_wh[:, :],
                                                   scalar=0.0, in1=ex[:, :],
                                                   op0=Alu.max, op1=Alu.add),
                   waits=[(asem, A_EX)])
            assert V.n == V_T1

            def ln_core(src, dst, a_idx, v_stats):
                V.emit(lambda: ve.bn_stats(out=st[:, :], in_=src[:, :]))
                V.emit(lambda: ve.bn_aggr(out=mv[:, :], in_=st[:, :]),
                       self_wait=True)
                assert V.n == v_stats + 1
                # scalar operands latch at dispatch; the asem wait (ACT wrote
                # rstd) transitively guarantees mv is long since drained
                V.emit(lambda: ve.tensor_scalar(out=dst[:, :], in0=src[:, :],
                                                scalar1=mv[:, 0:1],
                                                scalar2=rstd[:, 0:1],
                                                op0=Alu.subtract,
                                                op1=Alu.mult),
                       waits=[(asem, a_idx)])

            ln_core(t1, t2, A_R1, V_ST1)
            assert V.n == V_T2
            V.emit(lambda: ve.tensor_copy(out=t2T[:, :], in_=p_t2T[:, :]),
                   waits=[(psem, P_T2T)])
            # leaky(q2) = q2 - 0.8*min(q2, 0)
            V.emit(lambda: ve.tensor_scalar(out=lk1[:, :], in0=p_q2[:, :],
                                            scalar1=0.0, scalar2=0.8,
                                            op0=Alu.min, op1=Alu.mult),
                   waits=[(psem, P_Q2)])
            V.emit(lambda: ve.tensor_sub(out=t3[:, :], in0=p_q2[:, :],
                                         in1=lk1[:, :]))
            assert V.n == V_T3
            ln_core(t3, u, A_R2, V_ST2)
            assert V.n == V_U
            # t4 = u * nf_g + B
            V.emit(lambda: ve.tensor_mul(out=t4a[:, :], in0=u[:, :],
                                         in1=s_wB[:, B_NFG:B_NFG + 128]))
            V.emit(lambda: ve.tensor_add(out=t4[:, :], in0=t4a[:, :],
                                         in1=s_wB[:, B_NFB:B_NFB + 128]))
            V.emit(lambda: ve.tensor_copy(out=t4T[:, :], in_=p_t4T[:, :]),
                   waits=[(psem, P_T4T)])
            assert V.n == V_T4T
            # leaky with folded bias, per chunk (all three share one PSUM
            # bank: don't read before the PE wrote all of them — P10):
            #   y1T_c = mm_c - 0.8*min(mm_c + bb_c, 0)
            for c in range(3):
                bb_c = s_wB[:, B_BB3 + c:B_BB3 + c + 1]
                V.emit(lambda c=c, bb_c=bb_c: ve.tensor_scalar(
                    out=lka[:, c, :], in0=p_y1T[:, c, :],
                    scalar1=bb_c, scalar2=zeroc[:, 0:1],
                    op0=Alu.add, op1=Alu.min),
                    waits=[(psem, P_WL[2])] if c == 0 else ())
                V.emit(lambda c=c: ve.scalar_tensor_tensor(
                    out=y1T[:, c, :], in0=lka[:, c, :], scalar=-0.8,
                    in1=p_y1T[:, c, :], op0=Alu.mult, op1=Alu.add))
                assert V.n == V_Y1T[c]
            V.emit(lambda: ve.tensor_add(out=y3[:, :], in0=p_y2[:, :],
                                         in1=t4[:, :]),
                   waits=[(psem, P_Y2[2])])
            assert V.n == V_Y3
            ln_core(y3, y4, A_R3, V_ST3)
            assert V.n == V_Y4
            V.emit(lambda: ve.tensor_copy(out=y4T[:, :], in_=p_y4T[:, :]),
                   waits=[(psem, P_Y4T)])
            # out = p_o + wvb (a scalar constant, same for every partition)
            V.emit(lambda: ve.tensor_scalar_add(
                out=o_sb[:, :], in0=p_o[:, :],
                scalar1=s_wB[:, B_WVB:B_WVB + 1]),
                waits=[(psem, P_O)])
            assert V.n == V_OSB

    return nc, ctx


def _get_nc(validation=False):
    key = "ncv" if validation else "nc"
    if key not in _CACHE:
        _CACHE[key] = _build_nc(validation)
    return _CACHE[key][0]


def _prep_in_maps(inputs):
    """Host-side sharding + exact algebraic weight folding + packing."""
    g = lambda k: np.asarray(inputs[k], dtype=np.float64)
    x = g("x")
    ei = np.asarray(inputs["edge_index"]).astype(np.int64)
    W = g("W")
    ff_w, ff_b = g("ff_w"), g("ff_b")
    na_g, na_b = g("na_g"), g("na_b")
    nf_g, nf_b = g("nf_g"), g("nf_b")
    wl_w, wl_b = g("wl_w"), g("wl_b")
    w5_w, w5_b = g("w5_w"), g("w5_b")
    fn_g, fn_b = g("fn_g"), g("fn_b")
    wv_w, wv_b = g("wv_w"), g("wv_b")

    xj = x[ei[1]]                           # [E, D] gather on host
    ffw_eff = ff_w * na_g[None, :]          # fold LN(na) scale into ff
    ffb_eff = ff_b + ff_w @ na_b            # fold LN(na) bias into ff
    wv_eff = wv_w[0] * fn_g                 # fold LN(fn) scale into wv
    wvb_eff = wv_b[0] + wv_w[0] @ fn_b      # fold LN(fn) bias into wv
    # joint fold of wl_b and w5_b into the leaky shift bb and t4 bias B:
    #   bb = wl_b - wl_w @ (B - nf_b),  B - nf_b = w5_b + w5_w @ bb
    bb = np.linalg.solve(np.eye(3 * D) + wl_w @ w5_w, wl_b - wl_w @ w5_b)
    B_bias = nf_b + w5_b + w5_w @ bb

    wA = np.zeros((128, 256), np.float64)
    wA[:, A_ID:A_ID + 128] = np.eye(128)
    wA[0, A_FFB:A_FFB + 128] = ffb_eff

    wB = np.zeros((128, B_COLS), np.float64)
    wB[:, B_FFWT:B_FFWT + 128] = ffw_eff.T
    wB[:, B_WLWT:B_WLWT + 384] = wl_w.T
    wB[:, B_W5:B_W5 + 384] = w5_w.T.reshape(3, 128, 128).transpose(
        1, 0, 2).reshape(128, 384)
    wB[:, B_WVR:B_WVR + 128] = wv_eff[:, None]
    wB[:, B_NFG:B_NFG + 128] = nf_g[None, :]
    wB[:, B_NFB:B_NFB + 128] = B_bias[None, :]
    wB[:, B_BB3:B_BB3 + 3] = bb.reshape(3, 128).T
    wB[:, B_WVB] = wvb_eff

    f32 = lambda a: np.ascontiguousarray(a, dtype=np.float32)
    shared = {"wpacka": f32(wA), "wpackb": f32(wB)}
    in_maps = []
    for c in range(NCORES):
        xw = np.empty((128, 256), np.float64)
        xw[:, XW_XJT:XW_XJT + 128] = xj[c * PER:(c + 1) * PER].T
        xw[:, XW_W:XW_W + 128] = W
        m = dict(shared)
        m["xw"] = f32(xw)
        in_maps.append(m)
    return in_maps


def kernel(**inputs) -> np.ndarray:
    from concourse.bass_utils import run_bass_kernel_spmd

    nc = _get_nc()
    in_maps = _prep_in_maps(inputs)
    res = run_bass_kernel_spmd(nc, in_maps, core_ids=list(range(NCORES)))
    return np.concatenate(
        [np.asarray(res.results[c]["out"]).reshape(-1) for c in range(NCORES)]
    )


# revision 34
# speedup vs baseline: 1.6304x; 1.0560x over previous
"""Trainium2 Bass kernel for nn_AdjacencyGenerator (gnn_message_passing).

Math note (verified against the reference to ~5e-7 rel err):
  The reference builds att = softmax(..., axis=1) over an [E, E, D] tensor and
  then contracts it with einsum('ijk,il->ikl', att, Wh).  Since the j index
  appears only in att and softmax normalizes over j, sum_j att[i,j,k] == 1
  exactly, so h_prime[i,k,l] == Wh[i,l].  Every op after that point is
  row-wise over the [E*D, D] view, and row i*D+k of that view is Wh[i,:]
  independent of k.  The whole attention tensor therefore cancels and the
  output is a per-edge scalar o[i] = f(Wh[i,:]) repeated D times.

  f is: elu -> LN(na) -> ff linear -> leaky -> LN(nf) -> wl linear -> leaky
        -> w5 linear -> +residual -> LN(fn) -> wv linear.

  Exact algebraic folds used on the host (none are approximations):
    * na_g/na_b fold into ff_w/ff_b            (LN -> Linear)
    * fn_g/fn_b fold into wv_w/wv_b            (LN -> Linear)
    * wl_b and w5_b fold jointly into the leaky shift bb and the t4 bias B,
      solving (I + wl_w @ w5_w) bb = wl_b - wl_w @ w5_b on the host — this
      removes all wl/w5 bias matmuls exactly.
    * elu is computed as elu(x)+1 = exp(min(x,0)) + max(x,0); the +1 shift
      is constant along the normalized axis so the following LN cancels it.

  rstd(var) = exp(-0.5*ln(var+eps)) on the scalar engine: ln and exp live in
  the same ACT table set, so the whole kernel uses exactly one table load,
  pre-warmed off the critical path.

Distribution: shard the E=1024 edges 128 per core across 8 NeuronCores,
data-parallel; all weights replicated.  The edge gather x[edge_index[1]] is
part of input sharding, done on the host.  Inputs ship as three packed
images: [xjT|W] (per-core), [ident|ffb], and one [128, 1284] weight image.
"""

import numpy as np

D = 128
E = 1024
NCORES = 8
PER = E // NCORES  # 128 edges per core
EPS = 1e-5

# column offsets inside the packed images
XW_XJT, XW_W = 0, 128                      # d_xw [128, 256] (per-core)
A_ID, A_FFB = 0, 128                       # d_wA [128, 256]
B_FFWT, B_WLWT, B_W5, B_WVR, B_NFG, B_NFB, B_BB3, B_WVB = (
    0, 128, 512, 896, 1024, 1152, 1280, 1283)
B_COLS = 1284

_CACHE = {}


class _Seq:
    """Sequential instruction emitter for one engine with semaphore tags.

    attach=True (single-instruction ops, DVE/ACT): one wait rides on the
    instruction's own sync_info (HW allows a single attached wait); any
    extra waits are emitted standalone.  attach=False (multi-instruction
    groups like matmul, and DMA): all waits are standalone so they gate the
    whole group.
    """

    def __init__(self, eng, sem, all_self_waits, attach=False):
        self.eng, self.sem, self.n = eng, sem, 0
        self.all_self_waits = all_self_waits
        self.attach = attach

    def emit(self, make, waits=(), self_wait=False):
        allw = list(waits)
        if (self_wait or self.all_self_waits) and self.n:
            allw.append((self.sem, self.n))
        if self.attach and allw:
            for s, v in allw[:-1]:
                self.eng.wait_ge(s, v)
            inst = make()
            inst._wait_ge(*allw[-1])
        else:
            for s, v in allw:
                self.eng.wait_ge(s, v)
            inst = make()
        inst.then_inc(self.sem, 1)
        self.n += 1
        return self.n


def _build_nc(validation=False):
    import concourse.bass as bass
    from concourse import mybir

    f32 = mybir.dt.float32
    Alu = mybir.AluOpType
    Act = mybir.ActivationFunctionType

    nc = bass.Bass(detect_race_conditions=validation)

    d_xw = nc.dram_tensor("xw", [128, 256], f32, kind="ExternalInput")
    d_wA = nc.dram_tensor("wpacka", [128, 256], f32, kind="ExternalInput")
    d_wB = nc.dram_tensor("wpackb", [128, B_COLS], f32, kind="ExternalInput")
    d_out = nc.dram_tensor("out", [PER, D], f32, kind="ExternalOutput")

    from contextlib import ExitStack

    ctx = ExitStack()
    sb = lambda name, shape, dt=f32: ctx.enter_context(
        nc.sbuf_tensor(name, shape, dt))
    ps = lambda name, shape: ctx.enter_context(nc.psum_tensor(name, shape, f32))

    s_xw = sb("s_xw", [128, 256])
    s_wA = sb("s_wa", [128, 256])
    s_wB = sb("s_wb", [128, B_COLS])

    ones = sb("ones", [1, 128])
    epsc = sb("epsc", [PER, 1])
    zeroc = sb("zeroc", [PER, 1])
    m0 = sb("m0", [PER, D])        # min(Wh, 0)
    ex = sb("ex", [PER, D])        # exp(min(Wh, 0))
    t1 = sb("t1", [PER, D])        # elu(Wh) + 1
    t2 = sb("t2", [PER, D])        # LN1 core
    t2T = sb("t2t", [D, PER])
    lk1 = sb("lk1", [PER, D])
    t3 = sb("t3", [PER, D])        # leaky(ff out)
    u = sb("u", [PER, D])          # LN2 core
    t4a = sb("t4a", [PER, D])
    t4 = sb("t4", [PER, D])
    t4T = sb("t4t", [D, PER])
    lka = sb("lka", [128, 3, PER])
    y1T = sb("y1t", [128, 3, PER])
    y3 = sb("y3", [PER, D])
    y4 = sb("y4", [PER, D])        # LN3 core
    y4w = sb("y4w", [PER, D])
    ocol = sb("ocol", [PER, 1])
    zerot = sb("zerot", [PER, D])
    o_sb = sb("o_sb", [PER, D])
    st = sb("st", [PER, 6])        # LN scratch (reused by all three LNs)
    mv = sb("mv", [PER, 2])
    lnv = sb("lnv", [PER, 1])
    rstd = sb("rstd", [PER, 1])
    scr = sb("scr", [1, 1])        # ACT warmup scratch

    p_wh = ps("p_wh", [PER, D])
    p_t2T = ps("p_t2t", [D, PER])
    p_q2 = ps("p_q2", [PER, D])
    p_t4T = ps("p_t4t", [D, PER])
    p_y1T = ps("p_y1t", [128, 3, PER])
    p_y2 = ps("p_y2", [PER, D])

    dsem_x = ctx.enter_context(nc.semaphore("dsem_x"))
    dsem_a = ctx.enter_context(nc.semaphore("dsem_a"))
    dsem_b = ctx.enter_context(nc.semaphore("dsem_b"))
    psem = ctx.enter_context(nc.semaphore("psem"))
    vsem = ctx.enter_context(nc.semaphore("vsem"))
    asem = ctx.enter_context(nc.semaphore("asem"))
    gsem = ctx.enter_context(nc.semaphore("gsem"))

    # ---- vector op indices ----------------------------------------------
    V_M0, V_T1 = 1, 2
    V_ST1, V_MV1, V_T2 = 3, 4, 5
    V_T2T, V_LK1, V_T3 = 6, 7, 8
    V_ST2, V_MV2, V_U = 9, 10, 11
    V_T4A, V_T4, V_T4T = 12, 13, 14
    V_Y1T = [16, 18, 20]
    V_Y3 = 21
    V_ST3, V_MV3, V_Y4 = 22, 23, 24
    V_Y4W, V_OCOL, V_OSB = 25, 26, 27
    # ---- PE op indices ---------------------------------------------------
    P_WH, P_Q2B, P_T2T, P_Q2, P_T4T = 1, 2, 3, 4, 5
    P_WL = [6, 7, 8]
    P_Y2 = [9, 10, 11]
    # ---- ACT op indices --------------------------------------------------
    A_WARM, A_EX = 1, 2
    A_R1, A_R2, A_R3 = 4, 6, 8
    # ---- gpsimd ----------------------------------------------------------
    G_ONES, G_SETUP = 1, 4

    with nc.Block() as block:

        @block.sync
        def _(sync):
            sync.dma_start(out=s_xw[:, :], in_=d_xw[:, :]).then_inc(dsem_x, 16)
            sync.dma_start(out=s_wB[:, :], in_=d_wB[:, :]).then_inc(dsem_b, 16)
            sync.dma_start(out=s_wA[:, :], in_=d_wA[:, :]).then_inc(dsem_a, 16)
            sync.wait_ge(vsem, V_OSB)
            sync.dma_start(out=d_out[:, :], in_=o_sb[:, :]).then_inc(dsem_x, 16)
            sync.wait_ge(dsem_x, 32)

        @block.gpsimd
        def _(ge):
            ge.memset(ones[:, :], 1.0).then_inc(gsem, 1)
            ge.memset(epsc[:, :], EPS).then_inc(gsem, 1)
            ge.memset(zeroc[:, :], 0.0).then_inc(gsem, 1)
            ge.memset(zerot[:, :], 0.0).then_inc(gsem, 1)

        @block.scalar
        def _(se):
            A = _Seq(se, asem, validation, attach=True)
            # pre-warm the ln/exp table set off the critical path
            A.emit(lambda: se.activation(out=scr[:, :], in_=ones[0:1, 0:1],
                                         func=Act.Ln),
                   waits=[(gsem, G_ONES)])
            A.emit(lambda: se.activation(out=ex[:, :], in_=m0[:, :],
                                         func=Act.Exp),
                   waits=[(vsem, V_M0)])
            assert A.n == A_EX
            for a_idx, v_mv in ((A_R1, V_MV1), (A_R2, V_MV2), (A_R3, V_MV3)):
                # rstd = exp(-0.5 * ln(var + eps))
                A.emit(lambda v_mv=v_mv: se.activation(
                    out=lnv[:, :], in_=mv[:, 1:2], func=Act.Ln,
                    bias=epsc[:, 0:1]),
                    waits=[(vsem, v_mv)])
                A.emit(lambda: se.activation(out=rstd[:, :], in_=lnv[:, :],
                                             func=Act.Exp, scale=-0.5),
                       self_wait=True)
                assert A.n == a_idx

        @block.tensor
        def _(te):
            T = _Seq(te, psem, validation)
            # Wh = xj @ W
            T.emit(lambda: te.matmul(p_wh[:, :], s_xw[:, XW_XJT:XW_XJT + 128],
                                     s_xw[:, XW_W:XW_W + 128],
                                     start=True, stop=True),
                   waits=[(dsem_x, 16)])
            # ff bias early (its only deps are DMA + ones memset)
            T.emit(lambda: te.matmul(p_q2[:, :], ones[:, :],
                                     s_wA[0:1, A_FFB:A_FFB + 128],
                                     start=True, stop=False,
                                     skip_group_check=True),
                   waits=[(dsem_a, 16), (dsem_b, 16), (gsem, G_ONES)])
            T.emit(lambda: te.transpose(p_t2T[:, :], t2[:, :],
                                        s_wA[:, A_ID:A_ID + 128]),
                   waits=[(vsem, V_T2)])
            T.emit(lambda: te.matmul(p_q2[:, :], t2T[:, :],
                                     s_wB[:, B_FFWT:B_FFWT + 128],
                                     start=False, stop=True,
                                     skip_group_check=True),
                   waits=[(vsem, V_T2T)])
            T.emit(lambda: te.transpose(p_t4T[:, :], t4[:, :],
                                        s_wA[:, A_ID:A_ID + 128]),
                   waits=[(vsem, V_T4)])
            for c in range(3):
                T.emit(lambda c=c: te.matmul(
                    p_y1T[:, c, :],
                    s_wB[:, B_WLWT + c * 128:B_WLWT + (c + 1) * 128],
                    t4T[:, :], start=True, stop=True),
                    waits=[(vsem, V_T4T)] if c == 0 else ())
            for c in range(3):
                T.emit(lambda c=c: te.matmul(
                    p_y2[:, :], y1T[:, c, :],
                    s_wB[:, B_W5 + c * 128:B_W5 + (c + 1) * 128],
                    start=(c == 0), stop=(c == 2)),
                    waits=[(vsem, V_Y1T[c])])
            assert T.n == P_Y2[2]

        @block.vector
        def _(ve):
            V = _Seq(ve, vsem, validation, attach=True)
            V.emit(lambda: ve.tensor_scalar_min(out=m0[:, :], in0=p_wh[:, :],
                                                scalar1=0.0),
                   waits=[(psem, P_WH), (gsem, G_SETUP)])
            V.emit(lambda: ve.scalar_tensor_tensor(out=t1[:, :], in0=p_wh[:, :],
                                                   scalar=0.0, in1=ex[:, :],
                                                   op0=Alu.max, op1=Alu.add),
                   waits=[(asem, A_EX)])
            assert V.n == V_T1

            def ln_core(src, dst, a_idx, v_stats):
                V.emit(lambda: ve.bn_stats(out=st[:, :], in_=src[:, :]))
                V.emit(lambda: ve.bn_aggr(out=mv[:, :], in_=st[:, :]),
                       self_wait=True)
                assert V.n == v_stats + 1
                # scalar operands latch at dispatch; the asem wait (ACT wrote
                # rstd) transitively guarantees mv is long since drained
                V.emit(lambda: ve.tensor_scalar(out=dst[:, :], in0=src[:, :],
                                                scalar1=mv[:, 0:1],
                                                scalar2=rstd[:, 0:1],
                                                op0=Alu.subtract,
                                                op1=Alu.mult),
                       waits=[(asem, a_idx)])

            ln_core(t1, t2, A_R1, V_ST1)
            assert V.n == V_T2
            V.emit(lambda: ve.tensor_copy(out=t2T[:, :], in_=p_t2T[:, :]),
                   waits=[(psem, P_T2T)])
            # leaky(q2) = q2 - 0.8*min(q2, 0)
            V.emit(lambda: ve.tensor_scalar(out=lk1[:, :], in0=p_q2[:, :],
                                            scalar1=0.0, scalar2=0.8,
                                            op0=Alu.min, op1=Alu.mult),
                   waits=[(psem, P_Q2)])
            V.emit(lambda: ve.tensor_sub(out=t3[:, :], in0=p_q2[:, :],
                                         in1=lk1[:, :]))
            assert V.n == V_T3
            ln_core(t3, u, A_R2, V_ST2)
            assert V.n == V_U
            # t4 = u * nf_g + B
            V.emit(lambda: ve.tensor_mul(out=t4a[:, :], in0=u[:, :],
                                         in1=s_wB[:, B_NFG:B_NFG + 128]))
            V.emit(lambda: ve.tensor_add(out=t4[:, :], in0=t4a[:, :],
                                         in1=s_wB[:, B_NFB:B_NFB + 128]))
            V.emit(lambda: ve.tensor_copy(out=t4T[:, :], in_=p_t4T[:, :]),
                   waits=[(psem, P_T4T)])
            assert V.n == V_T4T
            # leaky with folded bias, per chunk (all three share one PSUM
            # bank: don't read before the PE wrote all of them — P10):
            #   y1T_c = mm_c - 0.8*min(mm_c + bb_c, 0)
            for c in range(3):
                bb_c = s_wB[:, B_BB3 + c:B_BB3 + c + 1]
                V.emit(lambda c=c, bb_c=bb_c: ve.tensor_scalar(
                    out=lka[:, c, :], in0=p_y1T[:, c, :],
                    scalar1=bb_c, scalar2=zeroc[:, 0:1],
                    op0=Alu.add, op1=Alu.min),
                    waits=[(psem, P_WL[2])] if c == 0 else ())
                V.emit(lambda c=c: ve.scalar_tensor_tensor(
                    out=y1T[:, c, :], in0=lka[:, c, :], scalar=-0.8,
                    in1=p_y1T[:, c, :], op0=Alu.mult, op1=Alu.add))
                assert V.n == V_Y1T[c]
            V.emit(lambda: ve.tensor_add(out=y3[:, :], in0=p_y2[:, :],
                                         in1=t4[:, :]),
                   waits=[(psem, P_Y2[2])])
            assert V.n == V_Y3
            ln_core(y3, y4, A_R3, V_ST3)
            assert V.n == V_Y4
            # out[e] = sum_k y4[e,k]*wv_eff[k] + wvb, broadcast along free dim
            V.emit(lambda: ve.tensor_mul(out=y4w[:, :], in0=y4[:, :],
                                         in1=s_wB[:, B_WVR:B_WVR + 128]))
            V.emit(lambda: ve.tensor_reduce(out=ocol[:, :], in_=y4w[:, :],
                                            axis=mybir.AxisListType.X,
                                            op=Alu.add))
            V.emit(lambda: ve.tensor_scalar(out=o_sb[:, :], in0=zerot[:, :],
                                            scalar1=ocol[:, 0:1],
                                            scalar2=s_wB[:, B_WVB:B_WVB + 1],
                                            op0=Alu.add, op1=Alu.add),
                   self_wait=True)
            assert V.n == V_OSB

    return nc, ctx


def _get_nc(validation=False):
    key = "ncv" if validation else "nc"
    if key not in _CACHE:
        _CACHE[key] = _build_nc(validation)
    return _CACHE[key][0]


def _prep_in_maps(inputs):
    """Host-side sharding + exact algebraic weight folding + packing."""
    g = lambda k: np.asarray(inputs[k], dtype=np.float64)
    x = g("x")
    ei = np.asarray(inputs["edge_index"]).astype(np.int64)
    W = g("W")
    ff_w, ff_b = g("ff_w"), g("ff_b")
    na_g, na_b = g("na_g"), g("na_b")
    nf_g, nf_b = g("nf_g"), g("nf_b")
    wl_w, wl_b = g("wl_w"), g("wl_b")
    w5_w, w5_b = g("w5_w"), g("w5_b")
    fn_g, fn_b = g("fn_g"), g("fn_b")
    wv_w, wv_b = g("wv_w"), g("wv_b")

    xj = x[ei[1]]                           # [E, D] gather on host
    ffw_eff = ff_w * na_g[None, :]          # fold LN(na) scale into ff
    ffb_eff = ff_b + ff_w @ na_b            # fold LN(na) bias into ff
    wv_eff = wv_w[0] * fn_g                 # fold LN(fn) scale into wv
    wvb_eff = wv_b[0] + wv_w[0] @ fn_b      # fold LN(fn) bias into wv
    # joint fold of wl_b and w5_b into the leaky shift bb and t4 bias B:
    #   bb = wl_b - wl_w @ (B - nf_b),  B - nf_b = w5_b + w5_w @ bb
    bb = np.linalg.solve(np.eye(3 * D) + wl_w @ w5_w, wl_b - wl_w @ w5_b)
    B_bias = nf_b + w5_b + w5_w @ bb

    wA = np.zeros((128, 256), np.float64)
    wA[:, A_ID:A_ID + 128] = np.eye(128)
    wA[0, A_FFB:A_FFB + 128] = ffb_eff

    wB = np.zeros((128, B_COLS), np.float64)
    wB[:, B_FFWT:B_FFWT + 128] = ffw_eff.T
    wB[:, B_WLWT:B_WLWT + 384] = wl_w.T
    wB[:, B_W5:B_W5 + 384] = w5_w.T.reshape(3, 128, 128).transpose(
        1, 0, 2).reshape(128, 384)
    wB[:, B_WVR:B_WVR + 128] = wv_eff[None, :]
    wB[:, B_NFG:B_NFG + 128] = nf_g[None, :]
    wB[:, B_NFB:B_NFB + 128] = B_bias[None, :]
    wB[:, B_BB3:B_BB3 + 3] = bb.reshape(3, 128).T
    wB[:, B_WVB] = wvb_eff

    f32 = lambda a: np.ascontiguousarray(a, dtype=np.float32)
    shared = {"wpacka": f32(wA), "wpackb": f32(wB)}
    in_maps = []
    for c in range(NCORES):
        xw = np.empty((128, 256), np.float64)
        xw[:, XW_XJT:XW_XJT + 128] = xj[c * PER:(c + 1) * PER].T
        xw[:, XW_W:XW_W + 128] = W
        m = dict(shared)
        m["xw"] = f32(xw)
        in_maps.append(m)
    return in_maps


def kernel(**inputs) -> np.ndarray:
    from concourse.bass_utils import run_bass_kernel_spmd

    nc = _get_nc()
    in_maps = _prep_in_maps(inputs)
    res = run_bass_kernel_spmd(nc, in_maps, core_ids=list(range(NCORES)))
    return np.concatenate(
        [np.asarray(res.results[c]["out"]).reshape(-1) for c in range(NCORES)]
    )


# revision 40
# speedup vs baseline: 1.6334x; 1.0018x over previous
"""Trainium2 Bass kernel for nn_AdjacencyGenerator (gnn_message_passing).

Math note (verified against the reference to ~5e-7 rel err):
  The reference builds att = softmax(..., axis=1) over an [E, E, D] tensor and
  then contracts it with einsum('ijk,il->ikl', att, Wh).  Since the j index
  appears only in att and softmax normalizes over j, sum_j att[i,j,k] == 1
  exactly, so h_prime[i,k,l] == Wh[i,l].  Every op after that point is
  row-wise over the [E*D, D] view, and row i*D+k of that view is Wh[i,:]
  independent of k.  The whole attention tensor therefore cancels and the
  output is a per-edge scalar o[i] = f(Wh[i,:]) repeated D times.

  f is: elu -> LN(na) -> ff linear -> leaky -> LN(nf) -> wl linear -> leaky
        -> w5 linear -> +residual -> LN(fn) -> wv linear.

  Exact algebraic folds used on the host (none are approximations):
    * na_g/na_b fold into ff_w/ff_b            (LN -> Linear)
    * fn_g/fn_b fold into wv_w/wv_b            (LN -> Linear)
    * wl_b and w5_b fold jointly into the leaky shift bb and the t4 bias B,
      solving (I + wl_w @ w5_w) bb = wl_b - wl_w @ w5_b on the host — this
      removes all wl/w5 bias matmuls exactly.
    * elu is computed as elu(x)+1 = exp(min(x,0)) + max(x,0); the +1 shift
      is constant along the normalized axis so the following LN cancels it.

  rstd(var) = exp(-0.5*ln(var+eps)) on the scalar engine: ln and exp live in
  the same ACT table set, so the whole kernel uses exactly one table load,
  pre-warmed off the critical path.

Distribution: shard the E=1024 edges 128 per core across 8 NeuronCores,
data-parallel; all weights replicated.  The edge gather x[edge_index[1]] is
part of input sharding, done on the host.  Inputs ship as three packed
images: [xjT|W] (per-core), [ident|ffb], and one [128, 1284] weight image.
"""

import numpy as np

D = 128
E = 1024
NCORES = 8
PER = E // NCORES  # 128 edges per core
EPS = 1e-5

# column offsets inside the packed images
XW_XJT, XW_W = 0, 128                      # d_xw [128, 256] (per-core)
A_ID, A_FFB = 0, 128                       # d_wA [128, 256]
B_FFWT, B_WLWT, B_W5, B_WVR, B_NFG, B_NFB, B_BB3, B_WVB = (
    0, 128, 512, 896, 1024, 1152, 1280, 1283)
B_COLS = 1284

_CACHE = {}


class _Seq:
    """Sequential instruction emitter for one engine with semaphore tags.

    attach=True (single-instruction ops, DVE/ACT): one wait rides on the
    instruction's own sync_info (HW allows a single attached wait); any
    extra waits are emitted standalone.  attach=False (multi-instruction
    groups like matmul, and DMA): all waits are standalone so they gate the
    whole group.
    """

    def __init__(self, eng, sem, all_self_waits, attach=False):
        self.eng, self.sem, self.n = eng, sem, 0
        self.all_self_waits = all_self_waits
        self.attach = attach

    def emit(self, make, waits=(), self_wait=False):
        allw = list(waits)
        if (self_wait or self.all_self_waits) and self.n:
            allw.append((self.sem, self.n))
        if self.attach and allw:
            for s, v in allw[:-1]:
                self.eng.wait_ge(s, v)
            inst = make()
            inst._wait_ge(*allw[-1])
        else:
            for s, v in allw:
                self.eng.wait_ge(s, v)
            inst = make()
        inst.then_inc(self.sem, 1)
        self.n += 1
        return self.n


def _build_nc(validation=False):
    import concourse.bass as bass
    from concourse import mybir

    f32 = mybir.dt.float32
    Alu = mybir.AluOpType
    Act = mybir.ActivationFunctionType

    nc = bass.Bass(detect_race_conditions=validation)

    d_xw = nc.dram_tensor("xw", [128, 256], f32, kind="ExternalInput")
    d_wA = nc.dram_tensor("wpacka", [128, 256], f32, kind="ExternalInput")
    d_wB = nc.dram_tensor("wpackb", [128, B_COLS], f32, kind="ExternalInput")
    d_out = nc.dram_tensor("out", [PER, D], f32, kind="ExternalOutput")

    from contextlib import ExitStack

    ctx = ExitStack()
    sb = lambda name, shape, dt=f32: ctx.enter_context(
        nc.sbuf_tensor(name, shape, dt))
    ps = lambda name, shape: ctx.enter_context(nc.psum_tensor(name, shape, f32))

    s_xw = sb("s_xw", [128, 256])
    s_wA = sb("s_wa", [128, 256])
    s_wB = sb("s_wb", [128, B_COLS])

    ones = sb("ones", [1, 128])
    epsc = sb("epsc", [PER, 1])
    zeroc = sb("zeroc", [PER, 1])
    m0 = sb("m0", [PER, D])        # min(Wh, 0)
    ex = sb("ex", [PER, D])        # exp(min(Wh, 0))
    t1 = sb("t1", [PER, D])        # elu(Wh) + 1
    t2 = sb("t2", [PER, D])        # LN1 core
    t2T = sb("t2t", [D, PER])
    lk1 = sb("lk1", [PER, D])
    t3 = sb("t3", [PER, D])        # leaky(ff out)
    u = sb("u", [PER, D])          # LN2 core
    t4a = sb("t4a", [PER, D])
    t4 = sb("t4", [PER, D])
    t4T = sb("t4t", [D, PER])
    lka = sb("lka", [128, 3, PER])
    y1T = sb("y1t", [128, 3, PER])
    y3 = sb("y3", [PER, D])
    y4 = sb("y4", [PER, D])        # LN3 core
    y4w = sb("y4w", [PER, D])
    ocol = sb("ocol", [PER, 1])
    zerot = sb("zerot", [PER, D])
    o_sb = sb("o_sb", [PER, D])
    st = sb("st", [PER, 6])        # LN scratch (reused by all three LNs)
    mv = sb("mv", [PER, 2])
    lnv = sb("lnv", [PER, 1])
    rstd = sb("rstd", [PER, 1])
    scr = sb("scr", [1, 1])        # ACT warmup scratch

    p_wh = ps("p_wh", [PER, D])
    p_t2T = ps("p_t2t", [D, PER])
    p_q2 = ps("p_q2", [PER, D])
    p_t4T = ps("p_t4t", [D, PER])
    p_y1T = ps("p_y1t", [128, 3, PER])
    p_y2 = ps("p_y2", [PER, D])

    dsem_x = ctx.enter_context(nc.semaphore("dsem_x"))
    dsem_a = ctx.enter_context(nc.semaphore("dsem_a"))
    dsem_b = ctx.enter_context(nc.semaphore("dsem_b"))
    psem = ctx.enter_context(nc.semaphore("psem"))
    vsem = ctx.enter_context(nc.semaphore("vsem"))
    asem = ctx.enter_context(nc.semaphore("asem"))
    gsem = ctx.enter_context(nc.semaphore("gsem"))

    # ---- vector op indices ----------------------------------------------
    V_M0, V_T1 = 1, 2
    V_ST1, V_MV1, V_T2 = 3, 4, 5
    V_T2T, V_LK1, V_T3 = 6, 7, 8
    V_ST2, V_MV2, V_U = 9, 10, 11
    V_T4A, V_T4, V_T4T = 12, 13, 14
    V_Y1T = [16, 18, 20]
    V_Y3 = 21
    V_ST3, V_MV3, V_Y4 = 22, 23, 24
    V_Y4W, V_OCOL, V_OSB = 25, 26, 27
    # ---- PE op indices ---------------------------------------------------
    P_WH, P_Q2B, P_T2T, P_Q2, P_T4T = 1, 2, 3, 4, 5
    P_WL = [6, 7, 8]
    P_Y2 = [9, 10, 11]
    # ---- ACT op indices --------------------------------------------------
    A_WARM, A_EX = 1, 2
    A_R1, A_R2, A_R3 = 4, 6, 8
    # ---- gpsimd ----------------------------------------------------------
    G_ONES, G_SETUP = 1, 4

    with nc.Block() as block:

        @block.sync
        def _(sync):
            sync.dma_start(out=s_xw[:, :], in_=d_xw[:, :]).then_inc(dsem_x, 16)
            sync.dma_start(out=s_wB[:, :], in_=d_wB[:, :]).then_inc(dsem_b, 16)
            sync.dma_start(out=s_wA[:, :], in_=d_wA[:, :]).then_inc(dsem_a, 16)
            sync.wait_ge(vsem, V_OSB)
            sync.dma_start(out=d_out[:, :], in_=o_sb[:, :]).then_inc(dsem_x, 16)
            sync.wait_ge(dsem_x, 32)

        @block.gpsimd
        def _(ge):
            ge.memset(ones[:, :], 1.0).then_inc(gsem, 1)
            ge.memset(epsc[:, :], EPS).then_inc(gsem, 1)
            ge.memset(zeroc[:, :], 0.0).then_inc(gsem, 1)
            ge.memset(zerot[:, :], 0.0).then_inc(gsem, 1)

        @block.scalar
        def _(se):
            A = _Seq(se, asem, validation, attach=True)
            # pre-warm the ln/exp table set off the critical path
            A.emit(lambda: se.activation(out=scr[:, :], in_=ones[0:1, 0:1],
                                         func=Act.Ln),
                   waits=[(gsem, G_ONES)])
            A.emit(lambda: se.activation(out=ex[:, :], in_=m0[:, :],
                                         func=Act.Exp),
                   waits=[(vsem, V_M0)])
            assert A.n == A_EX
            for a_idx, v_mv in ((A_R1, V_MV1), (A_R2, V_MV2), (A_R3, V_MV3)):
                # rstd = exp(-0.5 * ln(var + eps))
                A.emit(lambda v_mv=v_mv: se.activation(
                    out=lnv[:, :], in_=mv[:, 1:2], func=Act.Ln,
                    bias=epsc[:, 0:1]),
                    waits=[(vsem, v_mv)])
                A.emit(lambda: se.activation(out=rstd[:, :], in_=lnv[:, :],
                                             func=Act.Exp, scale=-0.5),
                       self_wait=True)
                assert A.n == a_idx

        @block.tensor
        def _(te):
            T = _Seq(te, psem, validation)
            # Wh = xj @ W
            T.emit(lambda: te.matmul(p_wh[:, :], s_xw[:, XW_XJT:XW_XJT + 128],
                                     s_xw[:, XW_W:XW_W + 128],
                                     start=True, stop=True),
                   waits=[(dsem_x, 16)])
            # ff bias early (its only deps are DMA + ones memset)
            T.emit(lambda: te.matmul(p_q2[:, :], ones[:, :],
                                     s_wA[0:1, A_FFB:A_FFB + 128],
                                     start=True, stop=False,
                                     skip_group_check=True),
                   waits=[(dsem_a, 16), (dsem_b, 16), (gsem, G_ONES)])
            T.emit(lambda: te.transpose(p_t2T[:, :], t2[:, :],
                                        s_wA[:, A_ID:A_ID + 128]),
                   waits=[(vsem, V_T2)])
            T.emit(lambda: te.matmul(p_q2[:, :], t2T[:, :],
                                     s_wB[:, B_FFWT:B_FFWT + 128],
                                     start=False, stop=True,
                                     skip_group_check=True),
                   waits=[(vsem, V_T2T)])
            T.emit(lambda: te.transpose(p_t4T[:, :], t4[:, :],
                                        s_wA[:, A_ID:A_ID + 128]),
                   waits=[(vsem, V_T4)])
            for c in range(3):
                T.emit(lambda c=c: te.matmul(
                    p_y1T[:, c, :],
                    s_wB[:, B_WLWT + c * 128:B_WLWT + (c + 1) * 128],
                    t4T[:, :], start=True, stop=True),
                    waits=[(vsem, V_T4T)] if c == 0 else ())
            for c in range(3):
                T.emit(lambda c=c: te.matmul(
                    p_y2[:, :], y1T[:, c, :],
                    s_wB[:, B_W5 + c * 128:B_W5 + (c + 1) * 128],
                    start=(c == 0), stop=(c == 2)),
                    waits=[(vsem, V_Y1T[c])])
            assert T.n == P_Y2[2]

        @block.vector
        def _(ve):
            V = _Seq(ve, vsem, validation, attach=True)
            V.emit(lambda: ve.tensor_scalar_min(out=m0[:, :], in0=p_wh[:, :],
                                                scalar1=0.0),
                   waits=[(psem, P_WH), (gsem, G_SETUP)])
            V.emit(lambda: ve.scalar_tensor_tensor(out=t1[:, :], in0=p_wh[:, :],
                                                   scalar=0.0, in1=ex[:, :],
                                                   op0=Alu.max, op1=Alu.add),
                   waits=[(asem, A_EX)])
            assert V.n == V_T1

            def ln_core(src, dst, a_idx, v_stats):
                V.emit(lambda: ve.bn_stats(out=st[:, :], in_=src[:, :]))
                V.emit(lambda: ve.bn_aggr(out=mv[:, :], in_=st[:, :]),
                       self_wait=True)
                assert V.n == v_stats + 1
                # scalar operands latch at dispatch; the asem wait (ACT wrote
                # rstd) transitively guarantees mv is long since drained
                V.emit(lambda: ve.tensor_scalar(out=dst[:, :], in0=src[:, :],
                                                scalar1=mv[:, 0:1],
                                                scalar2=rstd[:, 0:1],
                                                op0=Alu.subtract,
                                                op1=Alu.mult),
                       waits=[(asem, a_idx)])

            ln_core(t1, t2, A_R1, V_ST1)
            assert V.n == V_T2
            V.emit(lambda: ve.tensor_copy(out=t2T[:, :], in_=p_t2T[:, :]),
                   waits=[(psem, P_T2T)])
            # leaky(q2) = q2 - 0.8*min(q2, 0)
            V.emit(lambda: ve.tensor_scalar(out=lk1[:, :], in0=p_q2[:, :],
                                            scalar1=0.0, scalar2=0.8,
                                            op0=Alu.min, op1=Alu.mult),
                   waits=[(psem, P_Q2)])
            V.emit(lambda: ve.tensor_sub(out=t3[:, :], in0=p_q2[:, :],
                                         in1=lk1[:, :]))
            assert V.n == V_T3
            ln_core(t3, u, A_R2, V_ST2)
            assert V.n == V_U
            # t4 = u * nf_g + B
            V.emit(lambda: ve.tensor_mul(out=t4a[:, :], in0=u[:, :],
                                         in1=s_wB[:, B_NFG:B_NFG + 128]))
            V.emit(lambda: ve.tensor_add(out=t4[:, :], in0=t4a[:, :],
                                         in1=s_wB[:, B_NFB:B_NFB + 128]))
            V.emit(lambda: ve.tensor_copy(out=t4T[:, :], in_=p_t4T[:, :]),
                   waits=[(psem, P_T4T)])
            assert V.n == V_T4T
            # leaky with folded bias, per chunk (all three share one PSUM
            # bank: don't read before the PE wrote all of them — P10):
            #   y1T_c = mm_c - 0.8*min(mm_c + bb_c, 0)
            for c in range(3):
                bb_c = s_wB[:, B_BB3 + c:B_BB3 + c + 1]
                V.emit(lambda c=c, bb_c=bb_c: ve.tensor_scalar(
                    out=lka[:, c, :], in0=p_y1T[:, c, :],
                    scalar1=bb_c, scalar2=zeroc[:, 0:1],
                    op0=Alu.add, op1=Alu.min),
                    waits=[(psem, P_WL[2])] if c == 0 else ())
                V.emit(lambda c=c: ve.scalar_tensor_tensor(
                    out=y1T[:, c, :], in0=lka[:, c, :], scalar=-0.8,
                    in1=p_y1T[:, c, :], op0=Alu.mult, op1=Alu.add))
                assert V.n == V_Y1T[c]
            V.emit(lambda: ve.tensor_add(out=y3[:, :], in0=p_y2[:, :],
                                         in1=t4[:, :]),
                   waits=[(psem, P_Y2[2])])
            assert V.n == V_Y3
            ln_core(y3, y4, A_R3, V_ST3)
            assert V.n == V_Y4
            # out[e] = sum_k y4[e,k]*wv_eff[k] + wvb, broadcast along free dim
            V.emit(lambda: ve.tensor_mul(out=y4w[:, :], in0=y4[:, :],
                                         in1=s_wB[:, B_WVR:B_WVR + 128]))
            V.emit(lambda: ve.tensor_reduce(out=ocol[:, :], in_=y4w[:, :],
                                            axis=mybir.AxisListType.X,
                                            op=Alu.add))
            V.emit(lambda: ve.tensor_scalar(out=o_sb[:, :], in0=zerot[:, :],
                                            scalar1=ocol[:, 0:1],
                                            scalar2=s_wB[:, B_WVB:B_WVB + 1],
                                            op0=Alu.add, op1=Alu.add),
                   self_wait=True)
            assert V.n == V_OSB

    return nc, ctx


def _get_nc(validation=False):
    key = "ncv" if validation else "nc"
    if key not in _CACHE:
        _CACHE[key] = _build_nc(validation)
    return _CACHE[key][0]


def _prep_in_maps(inputs):
    """Host-side sharding + exact algebraic weight folding + packing."""
    g = lambda k: np.asarray(inputs[k], dtype=np.float64)
    x = g("x")
    ei = np.asarray(inputs["edge_index"]).astype(np.int64)
    W = g("W")
    ff_w, ff_b = g("ff_w"), g("ff_b")
    na_g, na_b = g("na_g"), g("na_b")
    nf_g, nf_b = g("nf_g"), g("nf_b")
    wl_w, wl_b = g("wl_w"), g("wl_b")
    w5_w, w5_b = g("w5_w"), g("w5_b")
    fn_g, fn_b = g("fn_g"), g("fn_b")
    wv_w, wv_b = g("wv_w"), g("wv_b")

    xj = x[ei[1]]                           # [E, D] gather on host
    ffw_eff = ff_w * na_g[None, :]          # fold LN(na) scale into ff
    ffb_eff = ff_b + ff_w @ na_b            # fold LN(na) bias into ff
    wv_eff = wv_w[0] * fn_g                 # fold LN(fn) scale into wv
    wvb_eff = wv_b[0] + wv_w[0] @ fn_b      # fold LN(fn) bias into wv
    # joint fold of wl_b and w5_b into the leaky shift bb and t4 bias B:
    #   bb = wl_b - wl_w @ (B - nf_b),  B - nf_b = w5_b + w5_w @ bb
    bb = np.linalg.solve(np.eye(3 * D) + wl_w @ w5_w, wl_b - wl_w @ w5_b)
    B_bias = nf_b + w5_b + w5_w @ bb

    wA = np.zeros((128, 256), np.float64)
    wA[:, A_ID:A_ID + 128] = np.eye(128)
    wA[0, A_FFB:A_FFB + 128] = ffb_eff

    wB = np.zeros((128, B_COLS), np.float64)
    wB[:, B_FFWT:B_FFWT + 128] = ffw_eff.T
    wB[:, B_WLWT:B_WLWT + 384] = wl_w.T
    wB[:, B_W5:B_W5 + 384] = w5_w.T.reshape(3, 128, 128).transpose(
        1, 0, 2).reshape(128, 384)
    wB[:, B_WVR:B_WVR + 128] = wv_eff[None, :]
    wB[:, B_NFG:B_NFG + 128] = nf_g[None, :]
    wB[:, B_NFB:B_NFB + 128] = B_bias[None, :]
    wB[:, B_BB3:B_BB3 + 3] = bb.reshape(3, 128).T
    wB[:, B_WVB] = wvb_eff

    f32 = lambda a: np.ascontiguousarray(a, dtype=np.float32)
    shared = {"wpacka": f32(wA), "wpackb": f32(wB)}
    in_maps = []
    for c in range(NCORES):
        xw = np.empty((128, 256), np.float64)
        xw[:, XW_XJT:XW_XJT + 128] = xj[c * PER:(c + 1) * PER].T
        xw[:, XW_W:XW_W + 128] = W
        m = dict(shared)
        m["xw"] = f32(xw)
        in_maps.append(m)
    return in_maps


def kernel(**inputs) -> np.ndarray:
    from concourse.bass_utils import run_bass_kernel_spmd

    nc = _get_nc()
    in_maps = _prep_in_maps(inputs)
    res = run_bass_kernel_spmd(nc, in_maps, core_ids=list(range(NCORES)))
    return np.concatenate(
        [np.asarray(res.results[c]["out"]).reshape(-1) for c in range(NCORES)]
    )


# revision 44
# speedup vs baseline: 1.6501x; 1.0102x over previous
"""Trainium2 Bass kernel for nn_AdjacencyGenerator (gnn_message_passing).

Math note (verified against the reference to ~5e-7 rel err):
  The reference builds att = softmax(..., axis=1) over an [E, E, D] tensor and
  then contracts it with einsum('ijk,il->ikl', att, Wh).  Since the j index
  appears only in att and softmax normalizes over j, sum_j att[i,j,k] == 1
  exactly, so h_prime[i,k,l] == Wh[i,l].  Every op after that point is
  row-wise over the [E*D, D] view, and row i*D+k of that view is Wh[i,:]
  independent of k.  The whole attention tensor therefore cancels and the
  output is a per-edge scalar o[i] = f(Wh[i,:]) repeated D times.

  f is: elu -> LN(na) -> ff linear -> leaky -> LN(nf) -> wl linear -> leaky
        -> w5 linear -> +residual -> LN(fn) -> wv linear.

  Exact algebraic folds used on the host (none are approximations):
    * na_g/na_b fold into ff_w/ff_b            (LN -> Linear)
    * fn_g/fn_b fold into wv_w/wv_b            (LN -> Linear)
    * wl_b and w5_b fold jointly into the leaky shift bb and the t4 bias B,
      solving (I + wl_w @ w5_w) bb = wl_b - wl_w @ w5_b on the host — this
      removes all wl/w5 bias matmuls exactly.
    * elu is computed as elu(x)+1 = exp(min(x,0)) + max(x,0); the +1 shift
      is constant along the normalized axis so the following LN cancels it.

  rstd(var) = exp(-0.5*ln(var+eps)) on the scalar engine: ln and exp live in
  the same ACT table set, so the whole kernel uses exactly one table load,
  pre-warmed off the critical path.

Distribution: shard the E=1024 edges 128 per core across 8 NeuronCores,
data-parallel; all weights replicated.  The edge gather x[edge_index[1]] is
part of input sharding, done on the host.  Inputs ship as three packed
images: [xjT|W] (per-core), [ident|ffb], and one [128, 1284] weight image.
"""

import numpy as np

D = 128
E = 1024
NCORES = 8
PER = E // NCORES  # 128 edges per core
EPS = 1e-5

# column offsets inside the packed images
XW_XJT, XW_W = 0, 128                      # d_xw [128, 256] (per-core)
A_ID, A_FFB = 0, 128                       # d_wA [128, 256]
B_FFWT, B_WLWT, B_W5, B_WVR, B_NFG, B_NFB, B_BB3, B_WVB = (
    0, 128, 512, 896, 1024, 1152, 1280, 1283)
B_COLS = 1284

_CACHE = {}


class _Seq:
    """Sequential instruction emitter for one engine with semaphore tags.

    attach=True (single-instruction ops, DVE/ACT): one wait rides on the
    instruction's own sync_info (HW allows a single attached wait); any
    extra waits are emitted standalone.  attach=False (multi-instruction
    groups like matmul, and DMA): all waits are standalone so they gate the
    whole group.
    """

    def __init__(self, eng, sem, all_self_waits, attach=False):
        self.eng, self.sem, self.n = eng, sem, 0
        self.all_self_waits = all_self_waits
        self.attach = attach

    def emit(self, make, waits=(), self_wait=False):
        allw = list(waits)
        if (self_wait or self.all_self_waits) and self.n:
            allw.append((self.sem, self.n))
        if self.attach and allw:
            for s, v in allw[:-1]:
                self.eng.wait_ge(s, v)
            inst = make()
            inst._wait_ge(*allw[-1])
        else:
            for s, v in allw:
                self.eng.wait_ge(s, v)
            inst = make()
        inst.then_inc(self.sem, 1)
        self.n += 1
        return self.n


def _build_nc(validation=False):
    import concourse.bass as bass
    from concourse import mybir

    f32 = mybir.dt.float32
    Alu = mybir.AluOpType
    Act = mybir.ActivationFunctionType

    nc = bass.Bass(detect_race_conditions=validation)

    d_xw = nc.dram_tensor("xw", [128, 256], f32, kind="ExternalInput")
    d_wA = nc.dram_tensor("wpacka", [128, 256], f32, kind="ExternalInput")
    d_wB = nc.dram_tensor("wpackb", [128, B_COLS], f32, kind="ExternalInput")
    d_out = nc.dram_tensor("out", [PER, D], f32, kind="ExternalOutput")

    from contextlib import ExitStack

    ctx = ExitStack()
    sb = lambda name, shape, dt=f32: ctx.enter_context(
        nc.sbuf_tensor(name, shape, dt))
    ps = lambda name, shape: ctx.enter_context(nc.psum_tensor(name, shape, f32))

    s_xw = sb("s_xw", [128, 256])
    s_wA = sb("s_wa", [128, 256])
    s_wB = sb("s_wb", [128, B_COLS])

    ones = sb("ones", [1, 128])
    epsc = sb("epsc", [PER, 1])
    zeroc = sb("zeroc", [PER, 1])
    m0 = sb("m0", [PER, D])        # min(Wh, 0)
    ex = sb("ex", [PER, D])        # exp(min(Wh, 0))
    t1 = sb("t1", [PER, D])        # elu(Wh) + 1
    t2 = sb("t2", [PER, D])        # LN1 core
    t2T = sb("t2t", [D, PER])
    lk1 = sb("lk1", [PER, D])
    t3 = sb("t3", [PER, D])        # leaky(ff out)
    u = sb("u", [PER, D])          # LN2 core
    t4a = sb("t4a", [PER, D])
    t4 = sb("t4", [PER, D])
    t4T = sb("t4t", [D, PER])
    lka = sb("lka", [128, 3, PER])
    y1T = sb("y1t", [128, 3, PER])
    y3 = sb("y3", [PER, D])
    y4 = sb("y4", [PER, D])        # LN3 core
    y4w = sb("y4w", [PER, D])
    ocol = sb("ocol", [PER, 1])
    zerot = sb("zerot", [PER, D])
    o_sb = sb("o_sb", [PER, D])
    st = sb("st", [PER, 6])        # LN scratch (reused by all three LNs)
    mv = sb("mv", [PER, 2])
    lnv = sb("lnv", [PER, 1])
    rstd = sb("rstd", [PER, 1])
    scr = sb("scr", [1, 1])        # ACT warmup scratch

    p_wh = ps("p_wh", [PER, D])
    p_t2T = ps("p_t2t", [D, PER])
    p_q2 = ps("p_q2", [PER, D])
    p_t4T = ps("p_t4t", [D, PER])
    p_y1T = [ps(f"p_y1t{c}", [128, PER]) for c in range(3)]
    p_y2 = ps("p_y2", [PER, D])

    dsem_x = ctx.enter_context(nc.semaphore("dsem_x"))
    dsem_y = ctx.enter_context(nc.semaphore("dsem_y"))
    dsem_a = ctx.enter_context(nc.semaphore("dsem_a"))
    dsem_b = ctx.enter_context(nc.semaphore("dsem_b"))
    psem = ctx.enter_context(nc.semaphore("psem"))
    vsem = ctx.enter_context(nc.semaphore("vsem"))
    asem = ctx.enter_context(nc.semaphore("asem"))
    gsem = ctx.enter_context(nc.semaphore("gsem"))

    # ---- vector op indices ----------------------------------------------
    V_M0, V_T1 = 1, 2
    V_ST1, V_MV1, V_T2 = 3, 4, 5
    V_T2T, V_LK1, V_T3 = 6, 7, 8
    V_ST2, V_MV2, V_U = 9, 10, 11
    V_T4A, V_T4, V_T4T = 12, 13, 14
    V_Y1T = [16, 18, 20]
    V_Y3 = 21
    V_ST3, V_MV3, V_Y4 = 22, 23, 24
    V_Y4W, V_OCOL, V_OSB = 25, 26, 27
    # ---- PE op indices ---------------------------------------------------
    P_WH, P_Q2B, P_T2T, P_Q2, P_T4T = 2, 3, 4, 5, 6
    P_WL = [7, 8, 9]
    P_Y2 = [10, 11, 12]
    # ---- ACT op indices --------------------------------------------------
    A_WARM, A_EX = 1, 2
    A_R1, A_R2, A_R3 = 4, 6, 8
    # ---- gpsimd ----------------------------------------------------------
    G_ONES, G_SETUP = 1, 4

    with nc.Block() as block:

        @block.sync
        def _(sync):
            sync.dma_start(out=s_xw[0:64, :], in_=d_xw[0:64, :]
                           ).then_inc(dsem_x, 16)
            sync.dma_start(out=s_xw[64:128, :], in_=d_xw[64:128, :]
                           ).then_inc(dsem_y, 16)
            sync.dma_start(out=s_wB[:, :], in_=d_wB[:, :]).then_inc(dsem_b, 16)
            sync.wait_ge(vsem, V_OSB)
            sync.dma_start(out=d_out[:, :], in_=o_sb[:, :]).then_inc(dsem_x, 16)
            sync.wait_ge(dsem_x, 32)

        @block.gpsimd
        def _(ge):
            ge.memset(ones[:, :], 1.0).then_inc(gsem, 1)
            ge.memset(epsc[:, :], EPS).then_inc(gsem, 1)
            ge.memset(zeroc[:, :], 0.0).then_inc(gsem, 1)
            ge.memset(zerot[:, :], 0.0).then_inc(gsem, 1)
            ge.dma_start(out=s_wA[:, :], in_=d_wA[:, :]).then_inc(dsem_a, 16)

        @block.scalar
        def _(se):
            A = _Seq(se, asem, validation, attach=True)
            # pre-warm the ln/exp table set off the critical path
            A.emit(lambda: se.activation(out=scr[:, :], in_=ones[0:1, 0:1],
                                         func=Act.Ln),
                   waits=[(gsem, G_ONES)])
            A.emit(lambda: se.activation(out=ex[:, :], in_=m0[:, :],
                                         func=Act.Exp),
                   waits=[(vsem, V_M0)])
            assert A.n == A_EX
            for a_idx, v_mv in ((A_R1, V_MV1), (A_R2, V_MV2), (A_R3, V_MV3)):
                # rstd = exp(-0.5 * ln(var + eps))
                A.emit(lambda v_mv=v_mv: se.activation(
                    out=lnv[:, :], in_=mv[:, 1:2], func=Act.Ln,
                    bias=epsc[:, 0:1]),
                    waits=[(vsem, v_mv)])
                A.emit(lambda: se.activation(out=rstd[:, :], in_=lnv[:, :],
                                             func=Act.Exp, scale=-0.5),
                       self_wait=True)
                assert A.n == a_idx

        @block.tensor
        def _(te):
            T = _Seq(te, psem, validation)
            # Wh = xj @ W, split along K so the first half overlaps the
            # second half's DMA
            T.emit(lambda: te.matmul(p_wh[:, :],
                                     s_xw[0:64, XW_XJT:XW_XJT + 128],
                                     s_xw[0:64, XW_W:XW_W + 128],
                                     start=True, stop=False,
                                     skip_group_check=True),
                   waits=[(dsem_x, 16)])
            T.emit(lambda: te.matmul(p_wh[:, :],
                                     s_xw[64:128, XW_XJT:XW_XJT + 128],
                                     s_xw[64:128, XW_W:XW_W + 128],
                                     start=False, stop=True,
                                     skip_group_check=True),
                   waits=[(dsem_y, 16)])
            # ff bias early (its only deps are DMA + ones memset)
            T.emit(lambda: te.matmul(p_q2[:, :], ones[:, :],
                                     s_wA[0:1, A_FFB:A_FFB + 128],
                                     start=True, stop=False,
                                     skip_group_check=True),
                   waits=[(dsem_a, 16), (dsem_b, 16), (gsem, G_ONES)])
            T.emit(lambda: te.transpose(p_t2T[:, :], t2[:, :],
                                        s_wA[:, A_ID:A_ID + 128]),
                   waits=[(vsem, V_T2)])
            T.emit(lambda: te.matmul(p_q2[:, :], t2T[:, :],
                                     s_wB[:, B_FFWT:B_FFWT + 128],
                                     start=False, stop=True,
                                     skip_group_check=True),
                   waits=[(vsem, V_T2T)])
            T.emit(lambda: te.transpose(p_t4T[:, :], t4[:, :],
                                        s_wA[:, A_ID:A_ID + 128]),
                   waits=[(vsem, V_T4)])
            for c in range(3):
                T.emit(lambda c=c: te.matmul(
                    p_y1T[c][:, :],
                    s_wB[:, B_WLWT + c * 128:B_WLWT + (c + 1) * 128],
                    t4T[:, :], start=True, stop=True),
                    waits=[(vsem, V_T4T)] if c == 0 else ())
            for c in range(3):
                T.emit(lambda c=c: te.matmul(
                    p_y2[:, :], y1T[:, c, :],
                    s_wB[:, B_W5 + c * 128:B_W5 + (c + 1) * 128],
                    start=(c == 0), stop=(c == 2)),
                    waits=[(vsem, V_Y1T[c])])
            assert T.n == P_Y2[2]

        @block.vector
        def _(ve):
            V = _Seq(ve, vsem, validation, attach=True)
            V.emit(lambda: ve.tensor_scalar_min(out=m0[:, :], in0=p_wh[:, :],
                                                scalar1=0.0),
                   waits=[(psem, P_WH), (gsem, G_SETUP)])
            V.emit(lambda: ve.scalar_tensor_tensor(out=t1[:, :], in0=p_wh[:, :],
                                                   scalar=0.0, in1=ex[:, :],
                                                   op0=Alu.max, op1=Alu.add),
                   waits=[(asem, A_EX)])
            assert V.n == V_T1

            def ln_core(src, dst, a_idx, v_stats):
                V.emit(lambda: ve.bn_stats(out=st[:, :], in_=src[:, :]))
                V.emit(lambda: ve.bn_aggr(out=mv[:, :], in_=st[:, :]),
                       self_wait=True)
                assert V.n == v_stats + 1
                # scalar operands latch at dispatch; the asem wait (ACT wrote
                # rstd) transitively guarantees mv is long since drained
                V.emit(lambda: ve.tensor_scalar(out=dst[:, :], in0=src[:, :],
                                                scalar1=mv[:, 0:1],
                                                scalar2=rstd[:, 0:1],
                                                op0=Alu.subtract,
                                                op1=Alu.mult),
                       waits=[(asem, a_idx)])

            ln_core(t1, t2, A_R1, V_ST1)
            assert V.n == V_T2
            V.emit(lambda: ve.tensor_copy(out=t2T[:, :], in_=p_t2T[:, :]),
                   waits=[(psem, P_T2T)])
            # leaky(q2) = q2 - 0.8*min(q2, 0)
            V.emit(lambda: ve.tensor_scalar(out=lk1[:, :], in0=p_q2[:, :],
                                            scalar1=0.0, scalar2=0.8,
                                            op0=Alu.min, op1=Alu.mult),
                   waits=[(psem, P_Q2)])
            V.emit(lambda: ve.tensor_sub(out=t3[:, :], in0=p_q2[:, :],
                                         in1=lk1[:, :]))
            assert V.n == V_T3
            ln_core(t3, u, A_R2, V_ST2)
            assert V.n == V_U
            # t4 = u * nf_g + B
            V.emit(lambda: ve.tensor_mul(out=t4a[:, :], in0=u[:, :],
                                         in1=s_wB[:, B_NFG:B_NFG + 128]))
            V.emit(lambda: ve.tensor_add(out=t4[:, :], in0=t4a[:, :],
                                         in1=s_wB[:, B_NFB:B_NFB + 128]))
            V.emit(lambda: ve.tensor_copy(out=t4T[:, :], in_=p_t4T[:, :]),
                   waits=[(psem, P_T4T)])
            assert V.n == V_T4T
            # leaky with folded bias, per chunk (each wl output has its
            # own PSUM bank, so chunk c can be read while the PE writes c+1):
            #   y1T_c = mm_c - 0.8*min(mm_c + bb_c, 0)
            for c in range(3):
                bb_c = s_wB[:, B_BB3 + c:B_BB3 + c + 1]
                V.emit(lambda c=c, bb_c=bb_c: ve.tensor_scalar(
                    out=lka[:, c, :], in0=p_y1T[c][:, :],
                    scalar1=bb_c, scalar2=zeroc[:, 0:1],
                    op0=Alu.add, op1=Alu.min),
                    waits=[(psem, P_WL[c])])
                V.emit(lambda c=c: ve.scalar_tensor_tensor(
                    out=y1T[:, c, :], in0=lka[:, c, :], scalar=-0.8,
                    in1=p_y1T[c][:, :], op0=Alu.mult, op1=Alu.add))
                assert V.n == V_Y1T[c]
            V.emit(lambda: ve.tensor_add(out=y3[:, :], in0=p_y2[:, :],
                                         in1=t4[:, :]),
                   waits=[(psem, P_Y2[2])])
            assert V.n == V_Y3
            # LN3 fused with the wv dot product:
            #   out[e] = rstd3[e] * sum_k ((y3-m3)[e,k]*wv_eff[k]) + wvb
            # the (y3-m)*wv part runs on DVE while ACT computes rstd3
            V.emit(lambda: ve.bn_stats(out=st[:, :], in_=y3[:, :]))
            V.emit(lambda: ve.bn_aggr(out=mv[:, :], in_=st[:, :]),
                   self_wait=True)
            assert V.n == V_MV3
            V.emit(lambda: ve.scalar_tensor_tensor(
                out=y4w[:, :], in0=y3[:, :], scalar=mv[:, 0:1],
                in1=s_wB[:, B_WVR:B_WVR + 128],
                op0=Alu.subtract, op1=Alu.mult),
                self_wait=True)
            V.emit(lambda: ve.tensor_reduce(out=ocol[:, :], in_=y4w[:, :],
                                            axis=mybir.AxisListType.X,
                                            op=Alu.add))
            V.emit(lambda: ve.tensor_scalar(out=ocol[:, :], in0=ocol[:, :],
                                            scalar1=rstd[:, 0:1],
                                            scalar2=s_wB[:, B_WVB:B_WVB + 1],
                                            op0=Alu.mult, op1=Alu.add),
                   waits=[(asem, A_R3)])
            V.emit(lambda: ve.tensor_scalar_add(out=o_sb[:, :],
                                                in0=zerot[:, :],
                                                scalar1=ocol[:, 0:1]),
                   self_wait=True)
            assert V.n == V_OSB

    return nc, ctx


def _get_nc(validation=False):
    key = "ncv" if validation else "nc"
    if key not in _CACHE:
        _CACHE[key] = _build_nc(validation)
    return _CACHE[key][0]


def _prep_in_maps(inputs):
    """Host-side sharding + exact algebraic weight folding + packing."""
    g = lambda k: np.asarray(inputs[k], dtype=np.float64)
    x = g("x")
    ei = np.asarray(inputs["edge_index"]).astype(np.int64)
    W = g("W")
    ff_w, ff_b = g("ff_w"), g("ff_b")
    na_g, na_b = g("na_g"), g("na_b")
    nf_g, nf_b = g("nf_g"), g("nf_b")
    wl_w, wl_b = g("wl_w"), g("wl_b")
    w5_w, w5_b = g("w5_w"), g("w5_b")
    fn_g, fn_b = g("fn_g"), g("fn_b")
    wv_w, wv_b = g("wv_w"), g("wv_b")

    xj = x[ei[1]]                           # [E, D] gather on host
    ffw_eff = ff_w * na_g[None, :]          # fold LN(na) scale into ff
    ffb_eff = ff_b + ff_w @ na_b            # fold LN(na) bias into ff
    wv_eff = wv_w[0] * fn_g                 # fold LN(fn) scale into wv
    wvb_eff = wv_b[0] + wv_w[0] @ fn_b      # fold LN(fn) bias into wv
    # joint fold of wl_b and w5_b into the leaky shift bb and t4 bias B:
    #   bb = wl_b - wl_w @ (B - nf_b),  B - nf_b = w5_b + w5_w @ bb
    bb = np.linalg.solve(np.eye(3 * D) + wl_w @ w5_w, wl_b - wl_w @ w5_b)
    B_bias = nf_b + w5_b + w5_w @ bb

    wA = np.zeros((128, 256), np.float64)
    wA[:, A_ID:A_ID + 128] = np.eye(128)
    wA[0, A_FFB:A_FFB + 128] = ffb_eff

    wB = np.zeros((128, B_COLS), np.float64)
    wB[:, B_FFWT:B_FFWT + 128] = ffw_eff.T
    wB[:, B_WLWT:B_WLWT + 384] = wl_w.T
    wB[:, B_W5:B_W5 + 384] = w5_w.T.reshape(3, 128, 128).transpose(
        1, 0, 2).reshape(128, 384)
    wB[:, B_WVR:B_WVR + 128] = wv_eff[None, :]
    wB[:, B_NFG:B_NFG + 128] = nf_g[None, :]
    wB[:, B_NFB:B_NFB + 128] = B_bias[None, :]
    wB[:, B_BB3:B_BB3 + 3] = bb.reshape(3, 128).T
    wB[:, B_WVB] = wvb_eff

    f32 = lambda a: np.ascontiguousarray(a, dtype=np.float32)
    shared = {"wpacka": f32(wA), "wpackb": f32(wB)}
    in_maps = []
    for c in range(NCORES):
        xw = np.empty((128, 256), np.float64)
        xw[:, XW_XJT:XW_XJT + 128] = xj[c * PER:(c + 1) * PER].T
        xw[:, XW_W:XW_W + 128] = W
        m = dict(shared)
        m["xw"] = f32(xw)
        in_maps.append(m)
    return in_maps


def kernel(**inputs) -> np.ndarray:
    from concourse.bass_utils import run_bass_kernel_spmd

    nc = _get_nc()
    in_maps = _prep_in_maps(inputs)
    res = run_bass_kernel_spmd(nc, in_maps, core_ids=list(range(NCORES)))
    return np.concatenate(
        [np.asarray(res.results[c]["out"]).reshape(-1) for c in range(NCORES)]
    )
